# revision 7
# baseline (speedup 1.0000x reference)
"""MultiHeadGAT Trainium2 kernel: 8-core batch-parallel, transposed-layout pipeline.

Math: for scores e = lrelu(s_i[n] + s_j[m]), softmax numerator
  p = exp(lrelu(s_i+s_j)) = e^{0.2 s_i} * max(e^{0.8 s_i} * e^{s_j}, e^{0.2 s_j})
The e^{0.2 s_i} row factor cancels in softmax, so on-device we only compute
  q[m, n] = adjT[m, n] * max(Wbc[m, n] * u[m], v[m])
with Wbc = broadcast(e^{0.8 s_i}) (n-varying), u = e^{s_j}, v = e^{0.2 s_j}
(per-partition scalars), which is one fused tensor_scalar (mult+max) plus one
tensor_tensor (mask) per tile. Attention output and row-sum Z come from one
PE matmul with lhsT = [ones | pad | Wh_head]; normalization 1/Z = exp(-ln(Z)).
"""

import sys

sys.path.insert(0, "/opt/trn_rl_repo")

import numpy as np

B, N, IN_DIM, H, HD = 8, 1024, 128, 8, 16
OUT_DIM = H * HD
EPS = 1e-5
NB = N // 128  # 8 m-blocks

_CACHE = {}


def _patch_act_tables():
    # Force one activation table set for the whole kernel: every function we
    # use (Exp, Ln, Copy, Square, Relu, Identity) lives in
    # natural_log_exp_and_others; emptying the other sets makes Bacc's
    # table-load inserter emit exactly one ACT_TABLE_LOAD instead of
    # thrashing between exp/ln/small sets (~2.5us per reload).
    import concourse.bacc as bacc
    import concourse.hw_specs as hw_specs
    if getattr(bacc, "_act_tables_patched", False):
        return
    orig = hw_specs.get_activation_tables

    def patched(arch):
        t = dict(orig(arch))
        keep = "natural_log_exp_and_others"
        return {k: (v if k == keep else set()) for k, v in t.items()}

    bacc.get_activation_tables = patched
    bacc._act_tables_patched = True


def _build_program():
    import concourse.bacc as bacc
    import concourse.mybir as mybir
    import concourse.tile as tile

    _patch_act_tables()

    F16 = mybir.dt.float16
    F32 = mybir.dt.float32
    AF = mybir.ActivationFunctionType
    OP = mybir.AluOpType

    nc = bacc.Bacc("TRN2", target_bir_lowering=False, debug=False, num_devices=8)

    # ---- I/O ----
    hT = nc.dram_tensor("hT", [128, N], F16, kind="ExternalInput")
    adjT = nc.dram_tensor("adjT", [128, NB * N], F16, kind="ExternalInput")
    wcat = nc.dram_tensor("wcat", [128, 128], F16, kind="ExternalInput")
    adst = nc.dram_tensor("adst", [128, H], F16, kind="ExternalInput")
    arep = nc.dram_tensor("arep", [128, H * 128], F16, kind="ExternalInput")
    sel = nc.dram_tensor("sel", [16, H * 128], F16, kind="ExternalInput")
    eye8 = nc.dram_tensor("eye8", [1, H * H], F32, kind="ExternalInput")
    e16 = nc.dram_tensor("e16", [H, 128], F16, kind="ExternalInput")
    w1c = nc.dram_tensor("w1c", [128, 2 * OUT_DIM], F16, kind="ExternalInput")
    b1c = nc.dram_tensor("b1c", [128, 2], F32, kind="ExternalInput")
    w2c = nc.dram_tensor("w2c", [128, 2 * OUT_DIM], F16, kind="ExternalInput")
    b2c = nc.dram_tensor("b2c", [128, 1], F32, kind="ExternalInput")
    g1c = nc.dram_tensor("g1c", [128, 1], F32, kind="ExternalInput")
    b1lc = nc.dram_tensor("b1lc", [128, 1], F32, kind="ExternalInput")
    g2c = nc.dram_tensor("g2c", [128, 1], F32, kind="ExternalInput")
    b2lc = nc.dram_tensor("b2lc", [128, 1], F32, kind="ExternalInput")
    outT = nc.dram_tensor("outT", [128, N], F32, kind="ExternalOutput")

    with tile.TileContext(nc) as tc:
        with (
            tc.tile_pool(name="const", bufs=1) as cpool,
            tc.tile_pool(name="big", bufs=1) as big,
            tc.tile_pool(name="work", bufs=2) as work,
            tc.tile_pool(name="mid", bufs=1) as mid,
            tc.tile_pool(name="rows", bufs=1) as rows,
        ):
            # ---- load everything ----
            hT_t = cpool.tile([128, N], F16)
            nc.gpsimd.dma_start(hT_t[:], hT[:])
            adjT_t = cpool.tile([128, NB * N], F16)
            nc.gpsimd.dma_start(adjT_t[:], adjT[:])
            wcat_t = cpool.tile([128, 128], F16)
            nc.gpsimd.dma_start(wcat_t[:], wcat[:])
            adst_t = cpool.tile([128, H], F16)
            nc.gpsimd.dma_start(adst_t[:], adst[:])
            arep_t = cpool.tile([128, H * 128], F16)
            nc.gpsimd.dma_start(arep_t[:], arep[:])
            sel_t = cpool.tile([16, H * 128], F16)
            nc.gpsimd.dma_start(sel_t[:], sel[:])
            eye8_t = cpool.tile([1, H * H], F32)
            nc.gpsimd.dma_start(eye8_t[:], eye8[:])
            e16_t = cpool.tile([H, 128], F16)
            nc.gpsimd.dma_start(e16_t[:], e16[:])
            w1_t = cpool.tile([128, 2 * OUT_DIM], F16)
            nc.gpsimd.dma_start(w1_t[:], w1c[:])
            b1_t = cpool.tile([128, 2], F32)
            nc.gpsimd.dma_start(b1_t[:], b1c[:])
            w2_t = cpool.tile([128, 2 * OUT_DIM], F16)
            nc.gpsimd.dma_start(w2_t[:], w2c[:])
            b2_t = cpool.tile([128, 1], F32)
            nc.gpsimd.dma_start(b2_t[:], b2c[:])
            g1_t = cpool.tile([128, 1], F32)
            nc.gpsimd.dma_start(g1_t[:], g1c[:])
            b1l_t = cpool.tile([128, 1], F32)
            nc.gpsimd.dma_start(b1l_t[:], b1lc[:])
            g2_t = cpool.tile([128, 1], F32)
            nc.gpsimd.dma_start(g2_t[:], g2c[:])
            b2l_t = cpool.tile([128, 1], F32)
            nc.gpsimd.dma_start(b2l_t[:], b2lc[:])

            onescol = cpool.tile([128, 1], F16)
            nc.vector.memset(onescol[:], 1.0)
            zbias = cpool.tile([128, 1], F32)
            nc.vector.memset(zbias[:], 1e-4)
            epsbias = cpool.tile([128, 1], F32)
            nc.vector.memset(epsbias[:], EPS)
            onesrow = cpool.tile([1, 128], F32)
            nc.vector.memset(onesrow[:], 1.0)

            # ---- phase 1: WhT, Wh_nat->aug, s-cols(u,v), Wbc ----
            whT_sb = big.tile([128, N], F16)     # Wh^T: [(h,d), n]
            whnat_sb = big.tile([128, NB * 128], F16)  # Wh natural per m-block
            aug = big.tile([128, NB * 384], F16)  # per (mb,h): [ones|0*31|Wh16]
            u_all = big.tile([128, NB * H], F32)
            v_all = big.tile([128, NB * H], F32)
            wbc = big.tile([128, H * N], F16)    # e^{0.8 s_i} bcast, per head

            with tc.tile_pool(name="ps1", bufs=3, space="PSUM") as ps1:
                whT_ps = ps1.tile([128, N], F32, tag="ps1")
                for ch in range(2):
                    nc.tensor.matmul(
                        whT_ps[:, ch * 512:(ch + 1) * 512], wcat_t[:],
                        hT_t[:, ch * 512:(ch + 1) * 512], start=True, stop=True,
                    )
                nc.scalar.activation(whT_sb[:], whT_ps[:], AF.Copy)

                # aug skeleton: zeros + ones columns
                nc.vector.memset(aug[:], 0.0)
                aug4 = aug[:].rearrange("p (m h c) -> p m h c", m=NB, h=H, c=48)
                nc.vector.memset(aug4[:, :, :, 0:1], 1.0)

                for mb in range(NB):
                    wn_ps = ps1.tile([128, 128], F32, tag="ps1")
                    nc.tensor.matmul(
                        wn_ps[:], hT_t[:, mb * 128:(mb + 1) * 128], wcat_t[:],
                        start=True, stop=True,
                    )
                    nc.scalar.activation(
                        whnat_sb[:, mb * 128:(mb + 1) * 128], wn_ps[:], AF.Copy
                    )
                    sc_ps = ps1.tile([128, H], F32, tag="ps1")
                    nc.tensor.matmul(
                        sc_ps[:], whT_sb[:, mb * 128:(mb + 1) * 128], adst_t[:],
                        start=True, stop=True,
                    )
                    nc.scalar.activation(
                        u_all[:, mb * H:(mb + 1) * H], sc_ps[:], AF.Exp, scale=0.8
                    )
                    nc.scalar.activation(
                        v_all[:, mb * H:(mb + 1) * H], sc_ps[:], AF.Exp, scale=0.2
                    )
                for mb in range(NB):
                    for hh in range(H):
                        nc.vector.tensor_copy(
                            aug[:, mb * 384 + hh * 48 + 32: mb * 384 + hh * 48 + 48],
                            whnat_sb[:, mb * 128 + hh * 16: mb * 128 + hh * 16 + 16],
                        )
                for hh in range(H):
                    wb_ps = ps1.tile([128, N], F32, tag="ps1")
                    for ch in range(2):
                        nc.tensor.matmul(
                            wb_ps[:, ch * 512:(ch + 1) * 512],
                            arep_t[:, hh * 128:(hh + 1) * 128],
                            whT_sb[:, ch * 512:(ch + 1) * 512],
                            start=True, stop=True,
                        )
                    nc.scalar.activation(
                        wbc[:, hh * N:(hh + 1) * N], wb_ps[:], AF.Exp, scale=0.8
                    )

            # ---- phase 2: attention ----
            stage_all = big.tile([16, H * N], F16)
            with (
                tc.tile_pool(name="ps48", bufs=2, space="PSUM") as ps48,
                tc.tile_pool(name="psacc", bufs=1, space="PSUM") as psacc,
                tc.tile_pool(name="psz", bufs=1, space="PSUM") as psz,
            ):
                asm_ps = psacc.tile([128, N], F32)
                zall_ps = psz.tile([H, N], F32)
                for hh in range(H):
                    q_all = work.tile([128, NB * N], F16, tag="q")
                    for mb in range(NB):
                        t1 = work.tile([128, N], F16, tag="t1")
                        nc.vector.tensor_scalar(
                            t1[:], wbc[:, hh * N:(hh + 1) * N],
                            u_all[:, mb * H + hh: mb * H + hh + 1],
                            1.0,
                            op0=OP.mult, op1=OP.max,
                        )
                        nc.vector.scalar_tensor_tensor(
                            q_all[:, mb * N:(mb + 1) * N],
                            adjT_t[:, mb * N:(mb + 1) * N],
                            v_all[:, mb * H + hh: mb * H + hh + 1],
                            t1[:], op0=OP.mult, op1=OP.mult,
                        )
                    at_ps = ps48.tile([48, N], F32)
                    for mb in range(NB):
                        for ch in range(2):
                            nc.tensor.matmul(
                                at_ps[:, ch * 512:(ch + 1) * 512],
                                aug[:, mb * 384 + hh * 48: mb * 384 + hh * 48 + 48],
                                q_all[:, mb * N + ch * 512: mb * N + ch * 512 + 512],
                                start=(mb == 0), stop=(mb == NB - 1),
                            )
                    nc.scalar.activation(
                        stage_all[:, hh * N:(hh + 1) * N], at_ps[32:48, :], AF.Copy
                    )
                    lnz = work.tile([1, N], F32, tag="lnz")
                    nc.scalar.activation(lnz[:], at_ps[0:1, :], AF.Ln, bias=zbias[0:1, :])
                    for ch in range(2):
                        nc.tensor.matmul(
                            zall_ps[:, ch * 512:(ch + 1) * 512],
                            eye8_t[0:1, hh * H:(hh + 1) * H],
                            lnz[0:1, ch * 512:(ch + 1) * 512],
                            start=(hh == 0), stop=(hh == H - 1),
                        )
                        nc.tensor.matmul(
                            asm_ps[:, ch * 512:(ch + 1) * 512],
                            sel_t[:, hh * 128:(hh + 1) * 128],
                            stage_all[:, hh * N + ch * 512: hh * N + ch * 512 + 512],
                            start=(hh == 0), stop=(hh == H - 1),
                        )

                zinv_all = work.tile([H, N], F16, tag="zinv")
                nc.scalar.activation(zinv_all[:], zall_ps[:], AF.Exp, scale=-1.0)
                stage_full = big.tile([128, N], F16)
                nc.scalar.activation(stage_full[:], asm_ps[:], AF.Copy)

            with tc.tile_pool(name="ps3", bufs=2, space="PSUM") as ps3:
                zbc_ps = ps3.tile([128, N], F32, tag="ps3")
                for ch in range(2):
                    nc.tensor.matmul(
                        zbc_ps[:, ch * 512:(ch + 1) * 512], e16_t[:],
                        zinv_all[:, ch * 512:(ch + 1) * 512], start=True, stop=True,
                    )

                # ---- chunked epilogue: normalize+residual, LN1, FFN, LN2 ----
                C = 512

                def cs(t, c):
                    return t[:, c * C:(c + 1) * C]

                zbcf = big.tile([128, N], F16)
                hh_t = big.tile([128, N], F16)
                x_res = big.tile([128, N], F16)

                def layernorm_T(x_in, g_col, b_col, out_tile, ps_pool, nm):
                    """Column-chunked transposed layernorm; stats via PE."""
                    x2 = mid.tile([128, N], F16, tag=f"x2{nm}")
                    for c in range(2):
                        nc.vector.tensor_tensor(
                            cs(x2, c), cs(x_in, c), cs(x_in, c), op=OP.mult
                        )
                    for c in range(2):
                        mu_ps = ps_pool.tile([1, C], F32, tag="psr")
                        msq_ps = ps_pool.tile([1, C], F32, tag="psr")
                        nc.tensor.matmul(mu_ps[:], onescol[:], cs(x_in, c),
                                         start=True, stop=True)
                        nc.tensor.matmul(msq_ps[:], onescol[:], cs(x2, c),
                                         start=True, stop=True)
                        mu = rows.tile([1, C], F32, tag=f"mu{c}")
                        nc.scalar.activation(mu[:], mu_ps[:], AF.Copy, scale=1.0 / 128)
                        msq = rows.tile([1, C], F32, tag=f"msq{c}")
                        nc.scalar.activation(msq[:], msq_ps[:], AF.Copy, scale=1.0 / 128)
                        mu2 = rows.tile([1, C], F32, tag=f"mu2{c}")
                        nc.scalar.activation(mu2[:], mu[:], AF.Square)
                        nc.vector.tensor_tensor(msq[:], msq[:], mu2[:], op=OP.subtract)
                        nc.scalar.activation(mu2[:], msq[:], AF.Ln, bias=epsbias[0:1, :])
                        rstd = rows.tile([1, C], F32, tag=f"rstd{c}")
                        nc.scalar.activation(rstd[:], mu2[:], AF.Exp, scale=-0.5)
                        a_ps = ps_pool.tile([128, C], F32, tag="psb")
                        m_ps = ps_pool.tile([128, C], F32, tag="psb")
                        nc.tensor.matmul(a_ps[:], onesrow[:], rstd[:],
                                         start=True, stop=True)
                        nc.tensor.matmul(m_ps[:], onesrow[:], mu[:],
                                         start=True, stop=True)
                        a_bc = mid.tile([128, C], F16, tag=f"abc{nm}{c}")
                        nc.scalar.activation(a_bc[:], a_ps[:], AF.Copy)
                        m_bc = mid.tile([128, C], F16, tag=f"mbc{nm}{c}")
                        nc.scalar.activation(m_bc[:], m_ps[:], AF.Copy)
                        t_ = mid.tile([128, C], F16, tag=f"lnt{nm}{c}")
                        nc.vector.tensor_tensor(t_[:], cs(x_in, c), m_bc[:],
                                                op=OP.subtract)
                        xn = mid.tile([128, C], F16, tag=f"lnxn{nm}{c}")
                        nc.vector.tensor_tensor(xn[:], t_[:], a_bc[:], op=OP.mult)
                        nc.vector.tensor_scalar(
                            cs(out_tile, c), xn[:], g_col[:], b_col[:],
                            op0=OP.mult, op1=OP.add,
                        )

                xc = big.tile([128, N], F16)
                y1s = big.tile([128, 2 * N], F16)
                y2b = big.tile([128, N], F16)
                z_res = big.tile([128, N], F16)
                outT_sb = big.tile([128, N], F32)

                for c in range(2):
                    nc.scalar.activation(cs(zbcf, c), cs(zbc_ps, c), AF.Copy)
                    nc.vector.tensor_tensor(cs(hh_t, c), cs(stage_full, c),
                                            cs(zbcf, c), op=OP.mult)
                    nc.vector.tensor_tensor(cs(x_res, c), cs(hh_t, c),
                                            cs(hT_t, c), op=OP.add)
                layernorm_T(x_res, g1_t, b1l_t, xc, ps3, "a")

                # FFN (chunked)
                for cb in range(2):
                    y1_ps = ps3.tile([128, N], F32, tag="ps3")
                    for c in range(2):
                        nc.tensor.matmul(
                            cs(y1_ps, c), w1_t[:, cb * 128:(cb + 1) * 128],
                            cs(xc, c), start=True, stop=True,
                        )
                        nc.scalar.activation(
                            y1s[:, cb * N + c * C: cb * N + (c + 1) * C],
                            cs(y1_ps, c), AF.Relu, bias=b1_t[:, cb:cb + 1],
                        )
                y2_ps = ps3.tile([128, N], F32, tag="ps3")
                for cb in range(2):
                    for c in range(2):
                        nc.tensor.matmul(
                            cs(y2_ps, c), w2_t[:, cb * 128:(cb + 1) * 128],
                            y1s[:, cb * N + c * C: cb * N + (c + 1) * C],
                            start=(cb == 0), stop=(cb == 1),
                        )
                for c in range(2):
                    nc.scalar.activation(cs(y2b, c), cs(y2_ps, c), AF.Identity,
                                         bias=b2_t[:])
                    nc.vector.tensor_tensor(cs(z_res, c), cs(y2b, c), cs(xc, c),
                                            op=OP.add)
                layernorm_T(z_res, g2_t, b2l_t, outT_sb, ps3, "b")
                nc.gpsimd.dma_start(outT[:], outT_sb[:])

    nc.compile()
    return nc


def _host_prep(h, adj_mask, W, a, ln1_g, ln1_b, w1, b1, w2, b2, ln2_g, ln2_b):
    f16 = np.float16
    f32 = np.float32
    # shared weights
    wcat = np.ascontiguousarray(np.transpose(np.asarray(W, f32), (1, 0, 2)).reshape(128, 128)).astype(f16)
    a = np.asarray(a, f32)
    a_src, a_dst = a[:, :HD], a[:, HD:]
    adst = np.zeros((128, H), f16)
    arep = np.zeros((128, H * 128), f16)
    for hh in range(H):
        adst[hh * HD:(hh + 1) * HD, hh] = a_dst[hh].astype(f16)
        arep[hh * HD:(hh + 1) * HD, hh * 128:(hh + 1) * 128] = (
            a_src[hh].astype(f16)[:, None]
        )
    sel = np.zeros((16, H * 128), f16)
    for hh in range(H):
        sel[np.arange(16), hh * 128 + hh * 16 + np.arange(16)] = 1.0
    eye8 = np.zeros((1, H * H), f32)
    eye8[0, np.arange(H) * H + np.arange(H)] = 1.0
    e16 = np.zeros((H, 128), f16)
    for hh in range(H):
        e16[hh, hh * 16:(hh + 1) * 16] = 1.0
    w1c = np.asarray(w1, f32).astype(f16)                      # [128, 256]
    b1c = np.asarray(b1, f32).reshape(2, 128).T.copy()          # [128, 2]
    w2c = np.ascontiguousarray(np.asarray(w2, f32).reshape(2, 128, 128).transpose(1, 0, 2).reshape(128, 256)).astype(f16)
    b2c = np.asarray(b2, f32).reshape(128, 1).copy()
    g1c = np.asarray(ln1_g, f32).reshape(128, 1).copy()
    b1lc = np.asarray(ln1_b, f32).reshape(128, 1).copy()
    g2c = np.asarray(ln2_g, f32).reshape(128, 1).copy()
    b2lc = np.asarray(ln2_b, f32).reshape(128, 1).copy()

    shared = dict(wcat=wcat, adst=adst, arep=arep, sel=sel, eye8=eye8, e16=e16,
                  w1c=w1c, b1c=b1c, w2c=w2c, b2c=b2c, g1c=g1c, b1lc=b1lc,
                  g2c=g2c, b2lc=b2lc)

    h = np.asarray(h, f32)
    adj = np.asarray(adj_mask)
    in_maps = []
    for b in range(B):
        hT = np.ascontiguousarray(h[b].T).astype(f16)                  # [128, 1024]
        adjT = np.ascontiguousarray(
            (adj[b] != 0).T.astype(f16).reshape(NB, 128, N).transpose(1, 0, 2).reshape(128, NB * N)
        )
        in_maps.append(dict(hT=hT, adjT=adjT, **shared))
    return in_maps


def kernel(**inputs):
    from concourse.bass_utils import run_bass_kernel_spmd

    if "nc" not in _CACHE:
        _CACHE["nc"] = _build_program()
    nc = _CACHE["nc"]

    in_maps = _host_prep(**inputs)
    res = run_bass_kernel_spmd(nc, in_maps, list(range(B)))
    out = np.empty((B, N, OUT_DIM), np.float32)
    for b in range(B):
        out[b] = res.results[b]["outT"].T
    return out


# revision 8
# speedup vs baseline: 1.4152x; 1.4152x over previous
"""MultiHeadGAT Trainium2 kernel: 8-core batch-parallel, transposed-layout pipeline.

Math: for scores e = lrelu(s_i[n] + s_j[m]), softmax numerator
  p = exp(lrelu(s_i+s_j)) = e^{0.2 s_i} * max(e^{0.8 s_i} * e^{s_j}, e^{0.2 s_j})
The e^{0.2 s_i} row factor cancels in softmax, so on-device we only compute
  q[m, n] = adjT[m, n] * max(Wbc[m, n] * u[m], v[m])
with Wbc = broadcast(e^{0.8 s_i}) (n-varying), u = e^{s_j}, v = e^{0.2 s_j}
(per-partition scalars), which is one fused tensor_scalar (mult+max) plus one
tensor_tensor (mask) per tile. Attention output and row-sum Z come from one
PE matmul with lhsT = [ones | pad | Wh_head]; normalization 1/Z = exp(-ln(Z)).
"""

import sys

sys.path.insert(0, "/opt/trn_rl_repo")

import numpy as np

B, N, IN_DIM, H, HD = 8, 1024, 128, 8, 16
OUT_DIM = H * HD
EPS = 1e-5
NB = N // 128  # 8 m-blocks

_CACHE = {}


def _patch_act_tables():
    # Force one activation table set for the whole kernel: every function we
    # use (Exp, Ln, Copy, Square, Relu, Identity) lives in
    # natural_log_exp_and_others; emptying the other sets makes Bacc's
    # table-load inserter emit exactly one ACT_TABLE_LOAD instead of
    # thrashing between exp/ln/small sets (~2.5us per reload).
    import concourse.bacc as bacc
    import concourse.hw_specs as hw_specs
    if getattr(bacc, "_act_tables_patched", False):
        return
    orig = hw_specs.get_activation_tables

    def patched(arch):
        t = dict(orig(arch))
        keep = "natural_log_exp_and_others"
        return {k: (v if k == keep else set()) for k, v in t.items()}

    bacc.get_activation_tables = patched
    bacc._act_tables_patched = True


def _build_program():
    import concourse.bacc as bacc
    import concourse.mybir as mybir
    import concourse.tile as tile

    _patch_act_tables()

    F16 = mybir.dt.float16
    F32 = mybir.dt.float32
    AF = mybir.ActivationFunctionType
    OP = mybir.AluOpType

    nc = bacc.Bacc("TRN2", target_bir_lowering=False, debug=False, num_devices=8)

    # ---- I/O ----
    hT = nc.dram_tensor("hT", [128, N], F16, kind="ExternalInput")
    adjT = nc.dram_tensor("adjT", [128, NB * N], F16, kind="ExternalInput")
    wcat = nc.dram_tensor("wcat", [128, 128], F16, kind="ExternalInput")
    adst = nc.dram_tensor("adst", [128, H], F16, kind="ExternalInput")
    arep = nc.dram_tensor("arep", [128, H * 128], F16, kind="ExternalInput")
    sel = nc.dram_tensor("sel", [16, H * 128], F16, kind="ExternalInput")
    eye8 = nc.dram_tensor("eye8", [1, H * H], F32, kind="ExternalInput")
    e16 = nc.dram_tensor("e16", [H, 128], F16, kind="ExternalInput")
    w1c = nc.dram_tensor("w1c", [128, 2 * OUT_DIM], F16, kind="ExternalInput")
    b1c = nc.dram_tensor("b1c", [128, 2], F32, kind="ExternalInput")
    w2c = nc.dram_tensor("w2c", [128, 2 * OUT_DIM], F16, kind="ExternalInput")
    b2c = nc.dram_tensor("b2c", [128, 1], F32, kind="ExternalInput")
    g1c = nc.dram_tensor("g1c", [128, 1], F32, kind="ExternalInput")
    b1lc = nc.dram_tensor("b1lc", [128, 1], F32, kind="ExternalInput")
    g2c = nc.dram_tensor("g2c", [128, 1], F32, kind="ExternalInput")
    b2lc = nc.dram_tensor("b2lc", [128, 1], F32, kind="ExternalInput")
    outT = nc.dram_tensor("outT", [128, N], F32, kind="ExternalOutput")

    with tile.TileContext(nc) as tc:
        with (
            tc.tile_pool(name="const", bufs=1) as cpool,
            tc.tile_pool(name="big", bufs=1) as big,
            tc.tile_pool(name="work", bufs=2) as work,
            tc.tile_pool(name="mid", bufs=1) as mid,
            tc.tile_pool(name="rows", bufs=1) as rows,
        ):
            # ---- load everything ----
            hT_t = cpool.tile([128, N], F16)
            nc.gpsimd.dma_start(hT_t[:], hT[:])
            adjT_t = cpool.tile([128, NB * N], F16)
            nc.gpsimd.dma_start(adjT_t[:], adjT[:])
            wcat_t = cpool.tile([128, 128], F16)
            nc.gpsimd.dma_start(wcat_t[:], wcat[:])
            adst_t = cpool.tile([128, H], F16)
            nc.gpsimd.dma_start(adst_t[:], adst[:])
            arep_t = cpool.tile([128, H * 128], F16)
            nc.gpsimd.dma_start(arep_t[:], arep[:])
            sel_t = cpool.tile([16, H * 128], F16)
            nc.gpsimd.dma_start(sel_t[:], sel[:])
            eye8_t = cpool.tile([1, H * H], F32)
            nc.gpsimd.dma_start(eye8_t[:], eye8[:])
            e16_t = cpool.tile([H, 128], F16)
            nc.gpsimd.dma_start(e16_t[:], e16[:])
            w1_t = cpool.tile([128, 2 * OUT_DIM], F16)
            nc.gpsimd.dma_start(w1_t[:], w1c[:])
            b1_t = cpool.tile([128, 2], F32)
            nc.gpsimd.dma_start(b1_t[:], b1c[:])
            w2_t = cpool.tile([128, 2 * OUT_DIM], F16)
            nc.gpsimd.dma_start(w2_t[:], w2c[:])
            b2_t = cpool.tile([128, 1], F32)
            nc.gpsimd.dma_start(b2_t[:], b2c[:])
            g1_t = cpool.tile([128, 1], F32)
            nc.gpsimd.dma_start(g1_t[:], g1c[:])
            b1l_t = cpool.tile([128, 1], F32)
            nc.gpsimd.dma_start(b1l_t[:], b1lc[:])
            g2_t = cpool.tile([128, 1], F32)
            nc.gpsimd.dma_start(g2_t[:], g2c[:])
            b2l_t = cpool.tile([128, 1], F32)
            nc.gpsimd.dma_start(b2l_t[:], b2lc[:])

            onescol = cpool.tile([128, 1], F16)
            nc.vector.memset(onescol[:], 1.0)
            zbias = cpool.tile([128, 1], F32)
            nc.vector.memset(zbias[:], 1e-4)
            epsbias = cpool.tile([128, 1], F32)
            nc.vector.memset(epsbias[:], EPS)
            onesrow = cpool.tile([1, 128], F32)
            nc.vector.memset(onesrow[:], 1.0)

            # ---- phase 1: WhT, Wh_nat->aug, s-cols(u,v), Wbc ----
            whT_sb = big.tile([128, N], F16)     # Wh^T: [(h,d), n]
            whnat_sb = big.tile([128, NB * 128], F16)  # Wh natural per m-block
            aug = big.tile([128, NB * 384], F16)  # per (mb,h): [ones|0*31|Wh16]
            u_all = big.tile([128, NB * H], F32)
            v_all = big.tile([128, NB * H], F32)
            wbc = big.tile([128, H * N], F16)    # e^{0.8 s_i} bcast, per head

            with tc.tile_pool(name="ps1", bufs=3, space="PSUM") as ps1:
                whT_ps = ps1.tile([128, N], F32, tag="ps1")
                for ch in range(2):
                    nc.tensor.matmul(
                        whT_ps[:, ch * 512:(ch + 1) * 512], wcat_t[:],
                        hT_t[:, ch * 512:(ch + 1) * 512], start=True, stop=True,
                    )
                nc.scalar.activation(whT_sb[:], whT_ps[:], AF.Copy)

                # aug skeleton: zeros + ones columns
                nc.vector.memset(aug[:], 0.0)
                aug4 = aug[:].rearrange("p (m h c) -> p m h c", m=NB, h=H, c=48)
                nc.vector.memset(aug4[:, :, :, 0:1], 1.0)

                for mb in range(NB):
                    wn_ps = ps1.tile([128, 128], F32, tag="ps1")
                    nc.tensor.matmul(
                        wn_ps[:], hT_t[:, mb * 128:(mb + 1) * 128], wcat_t[:],
                        start=True, stop=True,
                    )
                    nc.scalar.activation(
                        whnat_sb[:, mb * 128:(mb + 1) * 128], wn_ps[:], AF.Copy
                    )
                    sc_ps = ps1.tile([128, H], F32, tag="ps1")
                    nc.tensor.matmul(
                        sc_ps[:], whT_sb[:, mb * 128:(mb + 1) * 128], adst_t[:],
                        start=True, stop=True,
                    )
                    nc.scalar.activation(
                        u_all[:, mb * H:(mb + 1) * H], sc_ps[:], AF.Exp, scale=1.0
                    )
                    nc.scalar.activation(
                        v_all[:, mb * H:(mb + 1) * H], sc_ps[:], AF.Exp, scale=0.2
                    )
                for mb in range(NB):
                    for hh in range(H):
                        nc.vector.tensor_copy(
                            aug[:, mb * 384 + hh * 48 + 32: mb * 384 + hh * 48 + 48],
                            whnat_sb[:, mb * 128 + hh * 16: mb * 128 + hh * 16 + 16],
                        )
                for hh in range(H):
                    wb_ps = ps1.tile([128, N], F32, tag="ps1")
                    for ch in range(2):
                        nc.tensor.matmul(
                            wb_ps[:, ch * 512:(ch + 1) * 512],
                            arep_t[:, hh * 128:(hh + 1) * 128],
                            whT_sb[:, ch * 512:(ch + 1) * 512],
                            start=True, stop=True,
                        )
                    nc.scalar.activation(
                        wbc[:, hh * N:(hh + 1) * N], wb_ps[:], AF.Exp, scale=0.8
                    )

            # ---- phase 2: attention ----
            stage_all = big.tile([16, H * N], F16)
            with (
                tc.tile_pool(name="ps48", bufs=2, space="PSUM") as ps48,
                tc.tile_pool(name="psacc", bufs=1, space="PSUM") as psacc,
                tc.tile_pool(name="psz", bufs=1, space="PSUM") as psz,
            ):
                asm_ps = psacc.tile([128, N], F32)
                zall_ps = psz.tile([H, N], F32)
                for hh in range(H):
                    q_all = work.tile([128, NB * N], F16, tag="q")
                    for mb in range(NB):
                        t1 = work.tile([128, N], F16, tag="t1")
                        nc.vector.tensor_scalar(
                            t1[:], wbc[:, hh * N:(hh + 1) * N],
                            u_all[:, mb * H + hh: mb * H + hh + 1],
                            v_all[:, mb * H + hh: mb * H + hh + 1],
                            op0=OP.mult, op1=OP.max,
                        )
                        nc.vector.tensor_tensor(
                            q_all[:, mb * N:(mb + 1) * N], t1[:],
                            adjT_t[:, mb * N:(mb + 1) * N], op=OP.mult,
                        )
                    at_ps = ps48.tile([48, N], F32)
                    for mb in range(NB):
                        for ch in range(2):
                            nc.tensor.matmul(
                                at_ps[:, ch * 512:(ch + 1) * 512],
                                aug[:, mb * 384 + hh * 48: mb * 384 + hh * 48 + 48],
                                q_all[:, mb * N + ch * 512: mb * N + ch * 512 + 512],
                                start=(mb == 0), stop=(mb == NB - 1),
                            )
                    nc.scalar.activation(
                        stage_all[:, hh * N:(hh + 1) * N], at_ps[32:48, :], AF.Copy
                    )
                    lnz = work.tile([1, N], F32, tag="lnz")
                    nc.scalar.activation(lnz[:], at_ps[0:1, :], AF.Ln, bias=zbias[0:1, :])
                    for ch in range(2):
                        nc.tensor.matmul(
                            zall_ps[:, ch * 512:(ch + 1) * 512],
                            eye8_t[0:1, hh * H:(hh + 1) * H],
                            lnz[0:1, ch * 512:(ch + 1) * 512],
                            start=(hh == 0), stop=(hh == H - 1),
                        )
                        nc.tensor.matmul(
                            asm_ps[:, ch * 512:(ch + 1) * 512],
                            sel_t[:, hh * 128:(hh + 1) * 128],
                            stage_all[:, hh * N + ch * 512: hh * N + ch * 512 + 512],
                            start=(hh == 0), stop=(hh == H - 1),
                        )

                zinv_all = work.tile([H, N], F16, tag="zinv")
                nc.scalar.activation(zinv_all[:], zall_ps[:], AF.Exp, scale=-1.0)
                stage_full = big.tile([128, N], F16)
                nc.scalar.activation(stage_full[:], asm_ps[:], AF.Copy)

            with tc.tile_pool(name="ps3", bufs=2, space="PSUM") as ps3:
                zbc_ps = ps3.tile([128, N], F32, tag="ps3")
                for ch in range(2):
                    nc.tensor.matmul(
                        zbc_ps[:, ch * 512:(ch + 1) * 512], e16_t[:],
                        zinv_all[:, ch * 512:(ch + 1) * 512], start=True, stop=True,
                    )

                # ---- chunked epilogue: normalize+residual, LN1, FFN, LN2 ----
                C = 512

                def cs(t, c):
                    return t[:, c * C:(c + 1) * C]

                zbcf = big.tile([128, N], F16)
                hh_t = big.tile([128, N], F16)
                x_res = big.tile([128, N], F16)

                def layernorm_T(x_in, g_col, b_col, out_tile, ps_pool, nm):
                    """Column-chunked transposed layernorm; stats via PE."""
                    x2 = mid.tile([128, N], F16, tag=f"x2{nm}")
                    for c in range(2):
                        nc.vector.tensor_tensor(
                            cs(x2, c), cs(x_in, c), cs(x_in, c), op=OP.mult
                        )
                    for c in range(2):
                        mu_ps = ps_pool.tile([1, C], F32, tag="psr")
                        msq_ps = ps_pool.tile([1, C], F32, tag="psr")
                        nc.tensor.matmul(mu_ps[:], onescol[:], cs(x_in, c),
                                         start=True, stop=True)
                        nc.tensor.matmul(msq_ps[:], onescol[:], cs(x2, c),
                                         start=True, stop=True)
                        mu = rows.tile([1, C], F32, tag=f"mu{c}")
                        nc.scalar.activation(mu[:], mu_ps[:], AF.Copy, scale=1.0 / 128)
                        msq = rows.tile([1, C], F32, tag=f"msq{c}")
                        nc.scalar.activation(msq[:], msq_ps[:], AF.Copy, scale=1.0 / 128)
                        mu2 = rows.tile([1, C], F32, tag=f"mu2{c}")
                        nc.scalar.activation(mu2[:], mu[:], AF.Square)
                        nc.vector.tensor_tensor(msq[:], msq[:], mu2[:], op=OP.subtract)
                        nc.scalar.activation(mu2[:], msq[:], AF.Ln, bias=epsbias[0:1, :])
                        rstd = rows.tile([1, C], F32, tag=f"rstd{c}")
                        nc.scalar.activation(rstd[:], mu2[:], AF.Exp, scale=-0.5)
                        a_ps = ps_pool.tile([128, C], F32, tag="psb")
                        m_ps = ps_pool.tile([128, C], F32, tag="psb")
                        nc.tensor.matmul(a_ps[:], onesrow[:], rstd[:],
                                         start=True, stop=True)
                        nc.tensor.matmul(m_ps[:], onesrow[:], mu[:],
                                         start=True, stop=True)
                        a_bc = mid.tile([128, C], F16, tag=f"abc{nm}{c}")
                        nc.scalar.activation(a_bc[:], a_ps[:], AF.Copy)
                        m_bc = mid.tile([128, C], F16, tag=f"mbc{nm}{c}")
                        nc.scalar.activation(m_bc[:], m_ps[:], AF.Copy)
                        t_ = mid.tile([128, C], F16, tag=f"lnt{nm}{c}")
                        nc.vector.tensor_tensor(t_[:], cs(x_in, c), m_bc[:],
                                                op=OP.subtract)
                        xn = mid.tile([128, C], F16, tag=f"lnxn{nm}{c}")
                        nc.vector.tensor_tensor(xn[:], t_[:], a_bc[:], op=OP.mult)
                        nc.vector.tensor_scalar(
                            cs(out_tile, c), xn[:], g_col[:], b_col[:],
                            op0=OP.mult, op1=OP.add,
                        )

                xc = big.tile([128, N], F16)
                y1s = big.tile([128, 2 * N], F16)
                y2b = big.tile([128, N], F16)
                z_res = big.tile([128, N], F16)
                outT_sb = big.tile([128, N], F32)

                for c in range(2):
                    nc.scalar.activation(cs(zbcf, c), cs(zbc_ps, c), AF.Copy)
                    nc.vector.tensor_tensor(cs(hh_t, c), cs(stage_full, c),
                                            cs(zbcf, c), op=OP.mult)
                    nc.vector.tensor_tensor(cs(x_res, c), cs(hh_t, c),
                                            cs(hT_t, c), op=OP.add)
                layernorm_T(x_res, g1_t, b1l_t, xc, ps3, "a")

                # FFN (chunked)
                for cb in range(2):
                    y1_ps = ps3.tile([128, N], F32, tag="ps3")
                    for c in range(2):
                        nc.tensor.matmul(
                            cs(y1_ps, c), w1_t[:, cb * 128:(cb + 1) * 128],
                            cs(xc, c), start=True, stop=True,
                        )
                        nc.scalar.activation(
                            y1s[:, cb * N + c * C: cb * N + (c + 1) * C],
                            cs(y1_ps, c), AF.Relu, bias=b1_t[:, cb:cb + 1],
                        )
                y2_ps = ps3.tile([128, N], F32, tag="ps3")
                for cb in range(2):
                    for c in range(2):
                        nc.tensor.matmul(
                            cs(y2_ps, c), w2_t[:, cb * 128:(cb + 1) * 128],
                            y1s[:, cb * N + c * C: cb * N + (c + 1) * C],
                            start=(cb == 0), stop=(cb == 1),
                        )
                for c in range(2):
                    nc.scalar.activation(cs(y2b, c), cs(y2_ps, c), AF.Identity,
                                         bias=b2_t[:])
                    nc.vector.tensor_tensor(cs(z_res, c), cs(y2b, c), cs(xc, c),
                                            op=OP.add)
                layernorm_T(z_res, g2_t, b2l_t, outT_sb, ps3, "b")
                nc.gpsimd.dma_start(outT[:], outT_sb[:])

    nc.compile()
    return nc


def _host_prep(h, adj_mask, W, a, ln1_g, ln1_b, w1, b1, w2, b2, ln2_g, ln2_b):
    f16 = np.float16
    f32 = np.float32
    # shared weights
    wcat = np.ascontiguousarray(np.transpose(np.asarray(W, f32), (1, 0, 2)).reshape(128, 128)).astype(f16)
    a = np.asarray(a, f32)
    a_src, a_dst = a[:, :HD], a[:, HD:]
    adst = np.zeros((128, H), f16)
    arep = np.zeros((128, H * 128), f16)
    for hh in range(H):
        adst[hh * HD:(hh + 1) * HD, hh] = a_dst[hh].astype(f16)
        arep[hh * HD:(hh + 1) * HD, hh * 128:(hh + 1) * 128] = (
            a_src[hh].astype(f16)[:, None]
        )
    sel = np.zeros((16, H * 128), f16)
    for hh in range(H):
        sel[np.arange(16), hh * 128 + hh * 16 + np.arange(16)] = 1.0
    eye8 = np.zeros((1, H * H), f32)
    eye8[0, np.arange(H) * H + np.arange(H)] = 1.0
    e16 = np.zeros((H, 128), f16)
    for hh in range(H):
        e16[hh, hh * 16:(hh + 1) * 16] = 1.0
    w1c = np.asarray(w1, f32).astype(f16)                      # [128, 256]
    b1c = np.asarray(b1, f32).reshape(2, 128).T.copy()          # [128, 2]
    w2c = np.ascontiguousarray(np.asarray(w2, f32).reshape(2, 128, 128).transpose(1, 0, 2).reshape(128, 256)).astype(f16)
    b2c = np.asarray(b2, f32).reshape(128, 1).copy()
    g1c = np.asarray(ln1_g, f32).reshape(128, 1).copy()
    b1lc = np.asarray(ln1_b, f32).reshape(128, 1).copy()
    g2c = np.asarray(ln2_g, f32).reshape(128, 1).copy()
    b2lc = np.asarray(ln2_b, f32).reshape(128, 1).copy()

    shared = dict(wcat=wcat, adst=adst, arep=arep, sel=sel, eye8=eye8, e16=e16,
                  w1c=w1c, b1c=b1c, w2c=w2c, b2c=b2c, g1c=g1c, b1lc=b1lc,
                  g2c=g2c, b2lc=b2lc)

    h = np.asarray(h, f32)
    adj = np.asarray(adj_mask)
    in_maps = []
    for b in range(B):
        hT = np.ascontiguousarray(h[b].T).astype(f16)                  # [128, 1024]
        adjT = np.ascontiguousarray(
            (adj[b] != 0).T.astype(f16).reshape(NB, 128, N).transpose(1, 0, 2).reshape(128, NB * N)
        )
        in_maps.append(dict(hT=hT, adjT=adjT, **shared))
    return in_maps


def kernel(**inputs):
    from concourse.bass_utils import run_bass_kernel_spmd

    if "nc" not in _CACHE:
        _CACHE["nc"] = _build_program()
    nc = _CACHE["nc"]

    in_maps = _host_prep(**inputs)
    res = run_bass_kernel_spmd(nc, in_maps, list(range(B)))
    out = np.empty((B, N, OUT_DIM), np.float32)
    for b in range(B):
        out[b] = res.results[b]["outT"].T
    return out


# revision 9
# speedup vs baseline: 1.5906x; 1.1240x over previous
"""MultiHeadGAT Trainium2 kernel: 8-core batch-parallel, transposed-layout pipeline.

Math: for scores e = lrelu(s_i[n] + s_j[m]), softmax numerator
  p = exp(lrelu(s_i+s_j)) = e^{0.2 s_i} * max(e^{0.8 s_i} * e^{s_j}, e^{0.2 s_j})
The e^{0.2 s_i} row factor cancels in softmax, so on-device we only compute
  q[m, n] = adjT[m, n] * max(Wbc[m, n] * u[m], v[m])
with Wbc = broadcast(e^{0.8 s_i}) (n-varying), u = e^{s_j}, v = e^{0.2 s_j}
(per-partition scalars), which is one fused tensor_scalar (mult+max) plus one
tensor_tensor (mask) per tile. Attention output and row-sum Z come from one
PE matmul with lhsT = [ones | pad | Wh_head]; normalization 1/Z = exp(-ln(Z)).
"""

import sys

sys.path.insert(0, "/opt/trn_rl_repo")

import numpy as np

B, N, IN_DIM, H, HD = 8, 1024, 128, 8, 16
OUT_DIM = H * HD
EPS = 1e-5
NB = N // 128  # 8 m-blocks

_CACHE = {}


def _patch_act_tables():
    # Force one activation table set for the whole kernel: every function we
    # use (Exp, Ln, Copy, Square, Relu, Identity) lives in
    # natural_log_exp_and_others; emptying the other sets makes Bacc's
    # table-load inserter emit exactly one ACT_TABLE_LOAD instead of
    # thrashing between exp/ln/small sets (~2.5us per reload).
    import concourse.bacc as bacc
    import concourse.hw_specs as hw_specs
    if getattr(bacc, "_act_tables_patched", False):
        return
    orig = hw_specs.get_activation_tables

    def patched(arch):
        t = dict(orig(arch))
        keep = "natural_log_exp_and_others"
        return {k: (v if k == keep else set()) for k, v in t.items()}

    bacc.get_activation_tables = patched
    bacc._act_tables_patched = True


def _build_program():
    import concourse.bacc as bacc
    import concourse.mybir as mybir
    import concourse.tile as tile

    _patch_act_tables()

    F16 = mybir.dt.float16
    F32 = mybir.dt.float32
    AF = mybir.ActivationFunctionType
    OP = mybir.AluOpType

    nc = bacc.Bacc("TRN2", target_bir_lowering=False, debug=False, num_devices=8)

    # ---- I/O ----
    hT = nc.dram_tensor("hT", [128, N], F16, kind="ExternalInput")
    adjT = nc.dram_tensor("adjT", [128, NB * N], F16, kind="ExternalInput")
    wcat = nc.dram_tensor("wcat", [128, 128], F16, kind="ExternalInput")
    adst = nc.dram_tensor("adst", [128, H], F16, kind="ExternalInput")
    arep = nc.dram_tensor("arep", [128, H * 128], F16, kind="ExternalInput")
    sel = nc.dram_tensor("sel", [16, H * 128], F16, kind="ExternalInput")
    eye8 = nc.dram_tensor("eye8", [1, H * H], F32, kind="ExternalInput")
    e16 = nc.dram_tensor("e16", [H, 128], F16, kind="ExternalInput")
    w1c = nc.dram_tensor("w1c", [128, 2 * OUT_DIM], F16, kind="ExternalInput")
    b1c = nc.dram_tensor("b1c", [128, 2], F32, kind="ExternalInput")
    w2c = nc.dram_tensor("w2c", [128, 2 * OUT_DIM], F16, kind="ExternalInput")
    b2c = nc.dram_tensor("b2c", [128, 1], F32, kind="ExternalInput")
    g1c = nc.dram_tensor("g1c", [128, 1], F32, kind="ExternalInput")
    b1lc = nc.dram_tensor("b1lc", [128, 1], F32, kind="ExternalInput")
    g2c = nc.dram_tensor("g2c", [128, 1], F32, kind="ExternalInput")
    b2lc = nc.dram_tensor("b2lc", [128, 1], F32, kind="ExternalInput")
    outT = nc.dram_tensor("outT", [128, N], F32, kind="ExternalOutput")

    with tile.TileContext(nc) as tc:
        with (
            tc.tile_pool(name="const", bufs=1) as cpool,
            tc.tile_pool(name="big", bufs=1) as big,
            tc.tile_pool(name="work", bufs=2) as work,
            tc.tile_pool(name="mid", bufs=1) as mid,
            tc.tile_pool(name="rows", bufs=1) as rows,
        ):
            # ---- load everything ----
            hT_t = cpool.tile([128, N], F16)
            nc.gpsimd.dma_start(hT_t[:], hT[:])
            adjT_t = cpool.tile([128, NB * N], F16)
            nc.gpsimd.dma_start(adjT_t[:], adjT[:])
            wcat_t = cpool.tile([128, 128], F16)
            nc.gpsimd.dma_start(wcat_t[:], wcat[:])
            adst_t = cpool.tile([128, H], F16)
            nc.gpsimd.dma_start(adst_t[:], adst[:])
            arep_t = cpool.tile([128, H * 128], F16)
            nc.gpsimd.dma_start(arep_t[:], arep[:])
            sel_t = cpool.tile([16, H * 128], F16)
            nc.gpsimd.dma_start(sel_t[:], sel[:])
            eye8_t = cpool.tile([1, H * H], F32)
            nc.gpsimd.dma_start(eye8_t[:], eye8[:])
            e16_t = cpool.tile([H, 128], F16)
            nc.gpsimd.dma_start(e16_t[:], e16[:])
            w1_t = cpool.tile([128, 2 * OUT_DIM], F16)
            nc.gpsimd.dma_start(w1_t[:], w1c[:])
            b1_t = cpool.tile([128, 2], F32)
            nc.gpsimd.dma_start(b1_t[:], b1c[:])
            w2_t = cpool.tile([128, 2 * OUT_DIM], F16)
            nc.gpsimd.dma_start(w2_t[:], w2c[:])
            b2_t = cpool.tile([128, 1], F32)
            nc.gpsimd.dma_start(b2_t[:], b2c[:])
            g1_t = cpool.tile([128, 1], F32)
            nc.gpsimd.dma_start(g1_t[:], g1c[:])
            b1l_t = cpool.tile([128, 1], F32)
            nc.gpsimd.dma_start(b1l_t[:], b1lc[:])
            g2_t = cpool.tile([128, 1], F32)
            nc.gpsimd.dma_start(g2_t[:], g2c[:])
            b2l_t = cpool.tile([128, 1], F32)
            nc.gpsimd.dma_start(b2l_t[:], b2lc[:])

            onescol = cpool.tile([128, 1], F16)
            nc.vector.memset(onescol[:], 1.0)
            jmat = cpool.tile([128, 128], F16)
            nc.vector.memset(jmat[:], 1.0 / 128)
            zbias = cpool.tile([128, 1], F32)
            nc.vector.memset(zbias[:], 1e-4)
            epsbias = cpool.tile([128, 1], F32)
            nc.vector.memset(epsbias[:], EPS)
            onesrow = cpool.tile([1, 128], F32)
            nc.vector.memset(onesrow[:], 1.0)

            # ---- phase 1: WhT, Wh_nat->aug, s-cols(u,v), Wbc ----
            whT_sb = big.tile([128, N], F16)     # Wh^T: [(h,d), n]
            whnat_sb = big.tile([128, NB * 128], F16)  # Wh natural per m-block
            aug = big.tile([128, NB * 384], F16)  # per (mb,h): [ones|0*31|Wh16]
            u_all = big.tile([128, NB * H], F32)
            v_all = big.tile([128, NB * H], F32)
            wbc = big.tile([128, H * N], F16)    # e^{0.8 s_i} bcast, per head

            with tc.tile_pool(name="ps1", bufs=3, space="PSUM") as ps1:
                whT_ps = ps1.tile([128, N], F32, tag="ps1")
                for ch in range(2):
                    nc.tensor.matmul(
                        whT_ps[:, ch * 512:(ch + 1) * 512], wcat_t[:],
                        hT_t[:, ch * 512:(ch + 1) * 512], start=True, stop=True,
                    )
                nc.scalar.activation(whT_sb[:], whT_ps[:], AF.Copy)

                # aug skeleton: zeros + ones columns
                nc.vector.memset(aug[:], 0.0)
                aug4 = aug[:].rearrange("p (m h c) -> p m h c", m=NB, h=H, c=48)
                nc.vector.memset(aug4[:, :, :, 0:1], 1.0)

                for mb in range(NB):
                    sc_ps = ps1.tile([128, H], F32, tag="ps1")
                    nc.tensor.matmul(
                        sc_ps[:], whT_sb[:, mb * 128:(mb + 1) * 128], adst_t[:],
                        start=True, stop=True,
                    )
                    nc.scalar.activation(
                        u_all[:, mb * H:(mb + 1) * H], sc_ps[:], AF.Exp, scale=1.0
                    )
                    nc.scalar.activation(
                        v_all[:, mb * H:(mb + 1) * H], sc_ps[:], AF.Exp, scale=0.2
                    )
                for hh in range(H):
                    wb_ps = ps1.tile([128, N], F32, tag="ps1")
                    for ch in range(2):
                        nc.tensor.matmul(
                            wb_ps[:, ch * 512:(ch + 1) * 512],
                            arep_t[:, hh * 128:(hh + 1) * 128],
                            whT_sb[:, ch * 512:(ch + 1) * 512],
                            start=True, stop=True,
                        )
                    nc.scalar.activation(
                        wbc[:, hh * N:(hh + 1) * N], wb_ps[:], AF.Exp, scale=0.8
                    )
                for mb in range(NB):
                    wn_ps = ps1.tile([128, 128], F32, tag="ps1")
                    nc.tensor.matmul(
                        wn_ps[:], hT_t[:, mb * 128:(mb + 1) * 128], wcat_t[:],
                        start=True, stop=True,
                    )
                    nc.scalar.activation(
                        whnat_sb[:, mb * 128:(mb + 1) * 128], wn_ps[:], AF.Copy
                    )
                for mb in range(NB):
                    for hh in range(H):
                        nc.scalar.activation(
                            aug[:, mb * 384 + hh * 48 + 32: mb * 384 + hh * 48 + 48],
                            whnat_sb[:, mb * 128 + hh * 16: mb * 128 + hh * 16 + 16],
                            AF.Copy,
                        )

            # ---- phase 2: attention ----
            stage_all = big.tile([16, H * N], F16)
            with (
                tc.tile_pool(name="ps48", bufs=2, space="PSUM") as ps48,
                tc.tile_pool(name="psacc", bufs=1, space="PSUM") as psacc,
                tc.tile_pool(name="psz", bufs=1, space="PSUM") as psz,
            ):
                asm_ps = psacc.tile([128, N], F32)
                zall_ps = psz.tile([H, N], F32)
                for hh in range(H):
                    q_all = work.tile([128, NB * N], F16, tag="q")
                    for mb in range(NB):
                        t1 = work.tile([128, N], F16, tag="t1")
                        nc.vector.tensor_scalar(
                            t1[:], wbc[:, hh * N:(hh + 1) * N],
                            u_all[:, mb * H + hh: mb * H + hh + 1],
                            v_all[:, mb * H + hh: mb * H + hh + 1],
                            op0=OP.mult, op1=OP.max,
                        )
                        nc.vector.tensor_tensor(
                            q_all[:, mb * N:(mb + 1) * N], t1[:],
                            adjT_t[:, mb * N:(mb + 1) * N], op=OP.mult,
                        )
                    at_ps = ps48.tile([48, N], F32)
                    for mb in range(NB):
                        for ch in range(2):
                            nc.tensor.matmul(
                                at_ps[:, ch * 512:(ch + 1) * 512],
                                aug[:, mb * 384 + hh * 48: mb * 384 + hh * 48 + 48],
                                q_all[:, mb * N + ch * 512: mb * N + ch * 512 + 512],
                                start=(mb == 0), stop=(mb == NB - 1),
                            )
                    nc.scalar.activation(
                        stage_all[:, hh * N:(hh + 1) * N], at_ps[32:48, :], AF.Copy
                    )
                    lnz = work.tile([1, N], F32, tag="lnz")
                    nc.scalar.activation(lnz[:], at_ps[0:1, :], AF.Ln, bias=zbias[0:1, :])
                    for ch in range(2):
                        nc.tensor.matmul(
                            zall_ps[:, ch * 512:(ch + 1) * 512],
                            eye8_t[0:1, hh * H:(hh + 1) * H],
                            lnz[0:1, ch * 512:(ch + 1) * 512],
                            start=(hh == 0), stop=(hh == H - 1),
                        )
                        nc.tensor.matmul(
                            asm_ps[:, ch * 512:(ch + 1) * 512],
                            sel_t[:, hh * 128:(hh + 1) * 128],
                            stage_all[:, hh * N + ch * 512: hh * N + ch * 512 + 512],
                            start=(hh == 0), stop=(hh == H - 1),
                        )

                zinv_all = work.tile([H, N], F16, tag="zinv")
                nc.scalar.activation(zinv_all[:], zall_ps[:], AF.Exp, scale=-1.0)
                stage_full = big.tile([128, N], F16)
                nc.scalar.activation(stage_full[:], asm_ps[:], AF.Copy)

            with tc.tile_pool(name="ps3", bufs=2, space="PSUM") as ps3:
                zbc_ps = ps3.tile([128, N], F32, tag="ps3")
                for ch in range(2):
                    nc.tensor.matmul(
                        zbc_ps[:, ch * 512:(ch + 1) * 512], e16_t[:],
                        zinv_all[:, ch * 512:(ch + 1) * 512], start=True, stop=True,
                    )

                # ---- chunked epilogue: normalize+residual, LN1, FFN, LN2 ----
                C = 512

                def cs(t, c):
                    return t[:, c * C:(c + 1) * C]

                zbcf = big.tile([128, N], F16)
                hh_t = big.tile([128, N], F16)
                x_res = big.tile([128, N], F16)

                def layernorm_T(x_in, g_col, b_col, out_tile, ps_pool, nm):
                    """Column-chunked transposed layernorm; J=ones/128 matmul
                    produces mean / mean-square directly as broadcast tiles."""
                    x2 = mid.tile([128, N], F16, tag=f"x2{nm}")
                    for c in range(2):
                        nc.vector.tensor_tensor(
                            cs(x2, c), cs(x_in, c), cs(x_in, c), op=OP.mult
                        )
                    for c in range(2):
                        mu_ps = ps_pool.tile([128, C], F32, tag="psb")
                        ssq_ps = ps_pool.tile([128, C], F32, tag="psb")
                        nc.tensor.matmul(mu_ps[:], jmat[:], cs(x_in, c),
                                         start=True, stop=True)
                        nc.tensor.matmul(ssq_ps[:], jmat[:], cs(x2, c),
                                         start=True, stop=True)
                        mu_bc = mid.tile([128, C], F16, tag=f"mbc{nm}{c}")
                        nc.scalar.activation(mu_bc[:], mu_ps[:], AF.Copy)
                        ssq_bc = mid.tile([128, C], F16, tag=f"sbc{nm}{c}")
                        nc.scalar.activation(ssq_bc[:], ssq_ps[:], AF.Copy)
                        mu2 = mid.tile([128, C], F16, tag=f"m2{nm}{c}")
                        nc.vector.tensor_tensor(mu2[:], mu_bc[:], mu_bc[:], op=OP.mult)
                        var = mid.tile([128, C], F16, tag=f"va{nm}{c}")
                        nc.vector.tensor_tensor(var[:], ssq_bc[:], mu2[:],
                                                op=OP.subtract)
                        lnv = mid.tile([128, C], F16, tag=f"lv{nm}{c}")
                        nc.scalar.activation(lnv[:], var[:], AF.Ln, bias=epsbias[:])
                        rstd = mid.tile([128, C], F16, tag=f"rs{nm}{c}")
                        nc.scalar.activation(rstd[:], lnv[:], AF.Exp, scale=-0.5)
                        t_ = mid.tile([128, C], F16, tag=f"lnt{nm}{c}")
                        nc.vector.tensor_tensor(t_[:], cs(x_in, c), mu_bc[:],
                                                op=OP.subtract)
                        xn = mid.tile([128, C], F16, tag=f"lnxn{nm}{c}")
                        nc.vector.tensor_tensor(xn[:], t_[:], rstd[:], op=OP.mult)
                        nc.vector.tensor_scalar(
                            cs(out_tile, c), xn[:], g_col[:], b_col[:],
                            op0=OP.mult, op1=OP.add,
                        )

                xc = big.tile([128, N], F16)
                y1s = big.tile([128, 2 * N], F16)
                y2b = big.tile([128, N], F16)
                z_res = big.tile([128, N], F16)
                outT_sb = big.tile([128, N], F32)

                for c in range(2):
                    nc.scalar.activation(cs(zbcf, c), cs(zbc_ps, c), AF.Copy)
                    nc.vector.tensor_tensor(cs(hh_t, c), cs(stage_full, c),
                                            cs(zbcf, c), op=OP.mult)
                    nc.vector.tensor_tensor(cs(x_res, c), cs(hh_t, c),
                                            cs(hT_t, c), op=OP.add)
                layernorm_T(x_res, g1_t, b1l_t, xc, ps3, "a")

                # FFN (chunked)
                for cb in range(2):
                    y1_ps = ps3.tile([128, N], F32, tag="ps3")
                    for c in range(2):
                        nc.tensor.matmul(
                            cs(y1_ps, c), w1_t[:, cb * 128:(cb + 1) * 128],
                            cs(xc, c), start=True, stop=True,
                        )
                        nc.scalar.activation(
                            y1s[:, cb * N + c * C: cb * N + (c + 1) * C],
                            cs(y1_ps, c), AF.Relu, bias=b1_t[:, cb:cb + 1],
                        )
                y2_ps = ps3.tile([128, N], F32, tag="ps3")
                for cb in range(2):
                    for c in range(2):
                        nc.tensor.matmul(
                            cs(y2_ps, c), w2_t[:, cb * 128:(cb + 1) * 128],
                            y1s[:, cb * N + c * C: cb * N + (c + 1) * C],
                            start=(cb == 0), stop=(cb == 1),
                        )
                for c in range(2):
                    nc.scalar.activation(cs(y2b, c), cs(y2_ps, c), AF.Identity,
                                         bias=b2_t[:])
                    nc.vector.tensor_tensor(cs(z_res, c), cs(y2b, c), cs(xc, c),
                                            op=OP.add)
                layernorm_T(z_res, g2_t, b2l_t, outT_sb, ps3, "b")
                nc.gpsimd.dma_start(outT[:], outT_sb[:])

    nc.compile()
    return nc


def _host_prep(h, adj_mask, W, a, ln1_g, ln1_b, w1, b1, w2, b2, ln2_g, ln2_b):
    f16 = np.float16
    f32 = np.float32
    # shared weights
    wcat = np.ascontiguousarray(np.transpose(np.asarray(W, f32), (1, 0, 2)).reshape(128, 128)).astype(f16)
    a = np.asarray(a, f32)
    a_src, a_dst = a[:, :HD], a[:, HD:]
    adst = np.zeros((128, H), f16)
    arep = np.zeros((128, H * 128), f16)
    for hh in range(H):
        adst[hh * HD:(hh + 1) * HD, hh] = a_dst[hh].astype(f16)
        arep[hh * HD:(hh + 1) * HD, hh * 128:(hh + 1) * 128] = (
            a_src[hh].astype(f16)[:, None]
        )
    sel = np.zeros((16, H * 128), f16)
    for hh in range(H):
        sel[np.arange(16), hh * 128 + hh * 16 + np.arange(16)] = 1.0
    eye8 = np.zeros((1, H * H), f32)
    eye8[0, np.arange(H) * H + np.arange(H)] = 1.0
    e16 = np.zeros((H, 128), f16)
    for hh in range(H):
        e16[hh, hh * 16:(hh + 1) * 16] = 1.0
    w1c = np.asarray(w1, f32).astype(f16)                      # [128, 256]
    b1c = np.asarray(b1, f32).reshape(2, 128).T.copy()          # [128, 2]
    w2c = np.ascontiguousarray(np.asarray(w2, f32).reshape(2, 128, 128).transpose(1, 0, 2).reshape(128, 256)).astype(f16)
    b2c = np.asarray(b2, f32).reshape(128, 1).copy()
    g1c = np.asarray(ln1_g, f32).reshape(128, 1).copy()
    b1lc = np.asarray(ln1_b, f32).reshape(128, 1).copy()
    g2c = np.asarray(ln2_g, f32).reshape(128, 1).copy()
    b2lc = np.asarray(ln2_b, f32).reshape(128, 1).copy()

    shared = dict(wcat=wcat, adst=adst, arep=arep, sel=sel, eye8=eye8, e16=e16,
                  w1c=w1c, b1c=b1c, w2c=w2c, b2c=b2c, g1c=g1c, b1lc=b1lc,
                  g2c=g2c, b2lc=b2lc)

    h = np.asarray(h, f32)
    adj = np.asarray(adj_mask)
    in_maps = []
    for b in range(B):
        hT = np.ascontiguousarray(h[b].T).astype(f16)                  # [128, 1024]
        adjT = np.ascontiguousarray(
            (adj[b] != 0).T.astype(f16).reshape(NB, 128, N).transpose(1, 0, 2).reshape(128, NB * N)
        )
        in_maps.append(dict(hT=hT, adjT=adjT, **shared))
    return in_maps


def kernel(**inputs):
    from concourse.bass_utils import run_bass_kernel_spmd

    if "nc" not in _CACHE:
        _CACHE["nc"] = _build_program()
    nc = _CACHE["nc"]

    in_maps = _host_prep(**inputs)
    res = run_bass_kernel_spmd(nc, in_maps, list(range(B)))
    out = np.empty((B, N, OUT_DIM), np.float32)
    for b in range(B):
        out[b] = res.results[b]["outT"].T
    return out


# revision 10
# speedup vs baseline: 1.6844x; 1.0590x over previous
"""MultiHeadGAT Trainium2 kernel: 8-core batch-parallel, transposed-layout pipeline.

Math: for scores e = lrelu(s_i[n] + s_j[m]), softmax numerator
  p = exp(lrelu(s_i+s_j)) = e^{0.2 s_i} * max(e^{0.8 s_i} * e^{s_j}, e^{0.2 s_j})
The e^{0.2 s_i} row factor cancels in softmax, so on-device we only compute
  q[m, n] = adjT[m, n] * max(Wbc[m, n] * u[m], v[m])
with Wbc = broadcast(e^{0.8 s_i}) (n-varying), u = e^{s_j}, v = e^{0.2 s_j}
(per-partition scalars), which is one fused tensor_scalar (mult+max) plus one
tensor_tensor (mask) per tile. Attention output and row-sum Z come from one
PE matmul with lhsT = [ones | pad | Wh_head]; normalization 1/Z = exp(-ln(Z)).
"""

import sys

sys.path.insert(0, "/opt/trn_rl_repo")

import numpy as np

B, N, IN_DIM, H, HD = 8, 1024, 128, 8, 16
OUT_DIM = H * HD
EPS = 1e-5
NB = N // 128  # 8 m-blocks

_CACHE = {}


def _patch_act_tables():
    # Force one activation table set for the whole kernel: every function we
    # use (Exp, Ln, Copy, Square, Relu, Identity) lives in
    # natural_log_exp_and_others; emptying the other sets makes Bacc's
    # table-load inserter emit exactly one ACT_TABLE_LOAD instead of
    # thrashing between exp/ln/small sets (~2.5us per reload).
    import concourse.bacc as bacc
    import concourse.hw_specs as hw_specs
    if getattr(bacc, "_act_tables_patched", False):
        return
    orig = hw_specs.get_activation_tables

    def patched(arch):
        t = dict(orig(arch))
        keep = "natural_log_exp_and_others"
        return {k: (v if k == keep else set()) for k, v in t.items()}

    bacc.get_activation_tables = patched
    bacc._act_tables_patched = True


def _build_program():
    import concourse.bacc as bacc
    import concourse.mybir as mybir
    import concourse.tile as tile

    _patch_act_tables()

    F16 = mybir.dt.float16
    F32 = mybir.dt.float32
    AF = mybir.ActivationFunctionType
    OP = mybir.AluOpType

    nc = bacc.Bacc("TRN2", target_bir_lowering=False, debug=False, num_devices=8)

    # ---- I/O ----
    hT = nc.dram_tensor("hT", [128, N], F16, kind="ExternalInput")
    adjT = nc.dram_tensor("adjT", [128, NB * N], F16, kind="ExternalInput")
    wcat = nc.dram_tensor("wcat", [128, 128], F16, kind="ExternalInput")
    adst = nc.dram_tensor("adst", [128, H], F16, kind="ExternalInput")
    arep = nc.dram_tensor("arep", [128, H * 128], F16, kind="ExternalInput")
    sel = nc.dram_tensor("sel", [16, H * 128], F16, kind="ExternalInput")
    eye8 = nc.dram_tensor("eye8", [1, H * H], F32, kind="ExternalInput")
    e16 = nc.dram_tensor("e16", [H, 128], F16, kind="ExternalInput")
    w1c = nc.dram_tensor("w1c", [128, 2 * OUT_DIM], F16, kind="ExternalInput")
    b1c = nc.dram_tensor("b1c", [128, 2], F32, kind="ExternalInput")
    w2c = nc.dram_tensor("w2c", [128, 2 * OUT_DIM], F16, kind="ExternalInput")
    b2c = nc.dram_tensor("b2c", [128, 1], F32, kind="ExternalInput")
    g1c = nc.dram_tensor("g1c", [128, 1], F32, kind="ExternalInput")
    b1lc = nc.dram_tensor("b1lc", [128, 1], F32, kind="ExternalInput")
    g2c = nc.dram_tensor("g2c", [128, 1], F32, kind="ExternalInput")
    b2lc = nc.dram_tensor("b2lc", [128, 1], F32, kind="ExternalInput")
    outT = nc.dram_tensor("outT", [128, N], F32, kind="ExternalOutput")

    with tile.TileContext(nc) as tc:
        with (
            tc.tile_pool(name="const", bufs=1) as cpool,
            tc.tile_pool(name="big", bufs=1) as big,
            tc.tile_pool(name="work", bufs=2) as work,
            tc.tile_pool(name="mid", bufs=1) as mid,
            tc.tile_pool(name="rows", bufs=1) as rows,
        ):
            # ---- load everything ----
            hT_t = cpool.tile([128, N], F16)
            nc.sync.dma_start(hT_t[:], hT[:])
            wcat_t = cpool.tile([128, 128], F16)
            nc.sync.dma_start(wcat_t[:], wcat[:])
            adst_t = cpool.tile([128, H], F16)
            nc.sync.dma_start(adst_t[:], adst[:])
            arep_t = cpool.tile([128, H * 128], F16)
            nc.sync.dma_start(arep_t[:], arep[:])
            sel_t = cpool.tile([16, H * 128], F16)
            nc.sync.dma_start(sel_t[:], sel[:])
            eye8_t = cpool.tile([1, H * H], F32)
            nc.sync.dma_start(eye8_t[:], eye8[:])
            e16_t = cpool.tile([H, 128], F16)
            nc.sync.dma_start(e16_t[:], e16[:])
            w1_t = cpool.tile([128, 2 * OUT_DIM], F16)
            nc.sync.dma_start(w1_t[:], w1c[:])
            b1_t = cpool.tile([128, 2], F32)
            nc.sync.dma_start(b1_t[:], b1c[:])
            w2_t = cpool.tile([128, 2 * OUT_DIM], F16)
            nc.sync.dma_start(w2_t[:], w2c[:])
            b2_t = cpool.tile([128, 1], F32)
            nc.sync.dma_start(b2_t[:], b2c[:])
            g1_t = cpool.tile([128, 1], F32)
            nc.sync.dma_start(g1_t[:], g1c[:])
            b1l_t = cpool.tile([128, 1], F32)
            nc.sync.dma_start(b1l_t[:], b1lc[:])
            g2_t = cpool.tile([128, 1], F32)
            nc.sync.dma_start(g2_t[:], g2c[:])
            b2l_t = cpool.tile([128, 1], F32)
            nc.sync.dma_start(b2l_t[:], b2lc[:])

            adjT_t = cpool.tile([128, NB * N], F16)
            nc.sync.dma_start(adjT_t[:], adjT[:])

            onescol = cpool.tile([128, 1], F16)
            nc.vector.memset(onescol[:], 1.0)
            jmat = cpool.tile([128, 128], F16)
            nc.vector.memset(jmat[:], 1.0 / 128)
            zbias = cpool.tile([128, 1], F32)
            nc.vector.memset(zbias[:], 1e-4)
            epsbias = cpool.tile([128, 1], F32)
            nc.vector.memset(epsbias[:], EPS)
            onesrow = cpool.tile([1, 128], F32)
            nc.vector.memset(onesrow[:], 1.0)

            # ---- phase 1: WhT, Wh_nat->aug, s-cols(u,v), Wbc ----
            whT_sb = big.tile([128, N], F16)     # Wh^T: [(h,d), n]
            whnat_sb = big.tile([128, NB * 128], F16)  # Wh natural per m-block
            aug = big.tile([128, NB * 384], F16)  # per (mb,h): [ones|0*31|Wh16]
            u_all = big.tile([128, NB * H], F32)
            v_all = big.tile([128, NB * H], F32)
            wbc = big.tile([128, H * N], F16)    # e^{0.8 s_i} bcast, per head

            with tc.tile_pool(name="ps1", bufs=3, space="PSUM") as ps1:
                whT_ps = ps1.tile([128, N], F32, tag="ps1")
                for ch in range(2):
                    nc.tensor.matmul(
                        whT_ps[:, ch * 512:(ch + 1) * 512], wcat_t[:],
                        hT_t[:, ch * 512:(ch + 1) * 512], start=True, stop=True,
                    )
                nc.scalar.activation(whT_sb[:], whT_ps[:], AF.Copy)

                # aug skeleton: zeros + ones columns
                nc.vector.memset(aug[:], 0.0)
                aug4 = aug[:].rearrange("p (m h c) -> p m h c", m=NB, h=H, c=48)
                nc.vector.memset(aug4[:, :, :, 0:1], 1.0)

                for mb in range(NB):
                    sc_ps = ps1.tile([128, H], F32, tag="ps1")
                    nc.tensor.matmul(
                        sc_ps[:], whT_sb[:, mb * 128:(mb + 1) * 128], adst_t[:],
                        start=True, stop=True,
                    )
                    nc.scalar.activation(
                        u_all[:, mb * H:(mb + 1) * H], sc_ps[:], AF.Exp, scale=1.0
                    )
                    nc.scalar.activation(
                        v_all[:, mb * H:(mb + 1) * H], sc_ps[:], AF.Exp, scale=0.2
                    )
                for hh in range(H):
                    wb_ps = ps1.tile([128, N], F32, tag="ps1")
                    for ch in range(2):
                        nc.tensor.matmul(
                            wb_ps[:, ch * 512:(ch + 1) * 512],
                            arep_t[:, hh * 128:(hh + 1) * 128],
                            whT_sb[:, ch * 512:(ch + 1) * 512],
                            start=True, stop=True,
                        )
                    nc.scalar.activation(
                        wbc[:, hh * N:(hh + 1) * N], wb_ps[:], AF.Exp, scale=0.8
                    )
                for mb in range(NB):
                    wn_ps = ps1.tile([128, 128], F32, tag="ps1")
                    nc.tensor.matmul(
                        wn_ps[:], hT_t[:, mb * 128:(mb + 1) * 128], wcat_t[:],
                        start=True, stop=True,
                    )
                    nc.scalar.activation(
                        whnat_sb[:, mb * 128:(mb + 1) * 128], wn_ps[:], AF.Copy
                    )
                for mb in range(NB):
                    for hh in range(H):
                        nc.scalar.activation(
                            aug[:, mb * 384 + hh * 48 + 32: mb * 384 + hh * 48 + 48],
                            whnat_sb[:, mb * 128 + hh * 16: mb * 128 + hh * 16 + 16],
                            AF.Copy,
                        )

            # ---- phase 2: attention ----
            stage_all = big.tile([16, H * N], F16)
            with (
                tc.tile_pool(name="ps48", bufs=2, space="PSUM") as ps48,
                tc.tile_pool(name="psacc", bufs=1, space="PSUM") as psacc,
                tc.tile_pool(name="psz", bufs=1, space="PSUM") as psz,
            ):
                asm_ps = psacc.tile([128, N], F32)
                zall_ps = psz.tile([H, N], F32)
                for hh in range(H):
                    q_all = work.tile([128, NB * N], F16, tag="q")
                    t1 = work.tile([128, NB * N], F16, tag="t1")
                    for mb in range(NB):
                        nc.vector.tensor_scalar(
                            t1[:, mb * N:(mb + 1) * N],
                            wbc[:, hh * N:(hh + 1) * N],
                            u_all[:, mb * H + hh: mb * H + hh + 1],
                            v_all[:, mb * H + hh: mb * H + hh + 1],
                            op0=OP.mult, op1=OP.max,
                        )
                    for half in range(2):
                        nc.vector.tensor_tensor(
                            q_all[:, half * 4 * N:(half + 1) * 4 * N],
                            t1[:, half * 4 * N:(half + 1) * 4 * N],
                            adjT_t[:, half * 4 * N:(half + 1) * 4 * N], op=OP.mult,
                        )
                    at_ps = ps48.tile([48, N], F32)
                    for mb in range(NB):
                        for ch in range(2):
                            nc.tensor.matmul(
                                at_ps[:, ch * 512:(ch + 1) * 512],
                                aug[:, mb * 384 + hh * 48: mb * 384 + hh * 48 + 48],
                                q_all[:, mb * N + ch * 512: mb * N + ch * 512 + 512],
                                start=(mb == 0), stop=(mb == NB - 1),
                            )
                    nc.scalar.activation(
                        stage_all[:, hh * N:(hh + 1) * N], at_ps[32:48, :], AF.Copy
                    )
                    lnz = work.tile([1, N], F32, tag="lnz")
                    nc.scalar.activation(lnz[:], at_ps[0:1, :], AF.Ln, bias=zbias[0:1, :])
                    for ch in range(2):
                        nc.tensor.matmul(
                            zall_ps[:, ch * 512:(ch + 1) * 512],
                            eye8_t[0:1, hh * H:(hh + 1) * H],
                            lnz[0:1, ch * 512:(ch + 1) * 512],
                            start=(hh == 0), stop=(hh == H - 1),
                        )
                        nc.tensor.matmul(
                            asm_ps[:, ch * 512:(ch + 1) * 512],
                            sel_t[:, hh * 128:(hh + 1) * 128],
                            stage_all[:, hh * N + ch * 512: hh * N + ch * 512 + 512],
                            start=(hh == 0), stop=(hh == H - 1),
                        )

                zinv_all = work.tile([H, N], F16, tag="zinv")
                nc.scalar.activation(zinv_all[:], zall_ps[:], AF.Exp, scale=-1.0)
                stage_full = big.tile([128, N], F16)
                nc.scalar.activation(stage_full[:], asm_ps[:], AF.Copy)

            with tc.tile_pool(name="ps3", bufs=2, space="PSUM") as ps3:
                zbc_ps = ps3.tile([128, N], F32, tag="ps3")
                for ch in range(2):
                    nc.tensor.matmul(
                        zbc_ps[:, ch * 512:(ch + 1) * 512], e16_t[:],
                        zinv_all[:, ch * 512:(ch + 1) * 512], start=True, stop=True,
                    )

                # ---- chunked epilogue: normalize+residual, LN1, FFN, LN2 ----
                C = 512

                def cs(t, c):
                    return t[:, c * C:(c + 1) * C]

                zbcf = big.tile([128, N], F16)
                hh_t = big.tile([128, N], F16)
                x_res = big.tile([128, N], F16)

                def layernorm_T(x_in, g_col, b_col, out_tile, ps_pool, nm):
                    """Column-chunked transposed layernorm; J=ones/128 matmul
                    produces mean / mean-square directly as broadcast tiles."""
                    x2 = mid.tile([128, N], F16, tag=f"x2{nm}")
                    for c in range(2):
                        nc.vector.tensor_tensor(
                            cs(x2, c), cs(x_in, c), cs(x_in, c), op=OP.mult
                        )
                    for c in range(2):
                        mu_ps = ps_pool.tile([128, C], F32, tag="psb")
                        ssq_ps = ps_pool.tile([128, C], F32, tag="psb")
                        nc.tensor.matmul(mu_ps[:], jmat[:], cs(x_in, c),
                                         start=True, stop=True)
                        nc.tensor.matmul(ssq_ps[:], jmat[:], cs(x2, c),
                                         start=True, stop=True)
                        mu_bc = mid.tile([128, C], F16, tag=f"mbc{nm}{c}")
                        nc.scalar.activation(mu_bc[:], mu_ps[:], AF.Copy)
                        ssq_bc = mid.tile([128, C], F16, tag=f"sbc{nm}{c}")
                        nc.scalar.activation(ssq_bc[:], ssq_ps[:], AF.Copy)
                        mu2 = mid.tile([128, C], F16, tag=f"m2{nm}{c}")
                        nc.vector.tensor_tensor(mu2[:], mu_bc[:], mu_bc[:], op=OP.mult)
                        var = mid.tile([128, C], F16, tag=f"va{nm}{c}")
                        nc.vector.tensor_tensor(var[:], ssq_bc[:], mu2[:],
                                                op=OP.subtract)
                        lnv = mid.tile([128, C], F16, tag=f"lv{nm}{c}")
                        nc.scalar.activation(lnv[:], var[:], AF.Ln, bias=epsbias[:])
                        rstd = mid.tile([128, C], F16, tag=f"rs{nm}{c}")
                        nc.scalar.activation(rstd[:], lnv[:], AF.Exp, scale=-0.5)
                        t_ = mid.tile([128, C], F16, tag=f"lnt{nm}{c}")
                        nc.vector.tensor_tensor(t_[:], cs(x_in, c), mu_bc[:],
                                                op=OP.subtract)
                        xn = mid.tile([128, C], F16, tag=f"lnxn{nm}{c}")
                        nc.vector.tensor_tensor(xn[:], t_[:], rstd[:], op=OP.mult)
                        nc.vector.tensor_scalar(
                            cs(out_tile, c), xn[:], g_col[:], b_col[:],
                            op0=OP.mult, op1=OP.add,
                        )

                xc = big.tile([128, N], F16)
                y1s = big.tile([128, 2 * N], F16)
                y2b = big.tile([128, N], F16)
                z_res = big.tile([128, N], F16)
                outT_sb = big.tile([128, N], F32)

                for c in range(2):
                    nc.scalar.activation(cs(zbcf, c), cs(zbc_ps, c), AF.Copy)
                    nc.vector.tensor_tensor(cs(hh_t, c), cs(stage_full, c),
                                            cs(zbcf, c), op=OP.mult)
                    nc.vector.tensor_tensor(cs(x_res, c), cs(hh_t, c),
                                            cs(hT_t, c), op=OP.add)
                layernorm_T(x_res, g1_t, b1l_t, xc, ps3, "a")

                # FFN (chunked)
                for cb in range(2):
                    y1_ps = ps3.tile([128, N], F32, tag="ps3")
                    for c in range(2):
                        nc.tensor.matmul(
                            cs(y1_ps, c), w1_t[:, cb * 128:(cb + 1) * 128],
                            cs(xc, c), start=True, stop=True,
                        )
                        nc.scalar.activation(
                            y1s[:, cb * N + c * C: cb * N + (c + 1) * C],
                            cs(y1_ps, c), AF.Relu, bias=b1_t[:, cb:cb + 1],
                        )
                y2_ps = ps3.tile([128, N], F32, tag="ps3")
                for cb in range(2):
                    for c in range(2):
                        nc.tensor.matmul(
                            cs(y2_ps, c), w2_t[:, cb * 128:(cb + 1) * 128],
                            y1s[:, cb * N + c * C: cb * N + (c + 1) * C],
                            start=(cb == 0), stop=(cb == 1),
                        )
                for c in range(2):
                    nc.scalar.activation(cs(y2b, c), cs(y2_ps, c), AF.Identity,
                                         bias=b2_t[:])
                    nc.vector.tensor_tensor(cs(z_res, c), cs(y2b, c), cs(xc, c),
                                            op=OP.add)
                layernorm_T(z_res, g2_t, b2l_t, outT_sb, ps3, "b")
                for c in range(2):
                    nc.sync.dma_start(outT[:, c * C:(c + 1) * C],
                                      outT_sb[:, c * C:(c + 1) * C])

    nc.compile()
    return nc


def _host_prep(h, adj_mask, W, a, ln1_g, ln1_b, w1, b1, w2, b2, ln2_g, ln2_b):
    f16 = np.float16
    f32 = np.float32
    # shared weights
    wcat = np.ascontiguousarray(np.transpose(np.asarray(W, f32), (1, 0, 2)).reshape(128, 128)).astype(f16)
    a = np.asarray(a, f32)
    a_src, a_dst = a[:, :HD], a[:, HD:]
    adst = np.zeros((128, H), f16)
    arep = np.zeros((128, H * 128), f16)
    for hh in range(H):
        adst[hh * HD:(hh + 1) * HD, hh] = a_dst[hh].astype(f16)
        arep[hh * HD:(hh + 1) * HD, hh * 128:(hh + 1) * 128] = (
            a_src[hh].astype(f16)[:, None]
        )
    sel = np.zeros((16, H * 128), f16)
    for hh in range(H):
        sel[np.arange(16), hh * 128 + hh * 16 + np.arange(16)] = 1.0
    eye8 = np.zeros((1, H * H), f32)
    eye8[0, np.arange(H) * H + np.arange(H)] = 1.0
    e16 = np.zeros((H, 128), f16)
    for hh in range(H):
        e16[hh, hh * 16:(hh + 1) * 16] = 1.0
    w1c = np.asarray(w1, f32).astype(f16)                      # [128, 256]
    b1c = np.asarray(b1, f32).reshape(2, 128).T.copy()          # [128, 2]
    w2c = np.ascontiguousarray(np.asarray(w2, f32).reshape(2, 128, 128).transpose(1, 0, 2).reshape(128, 256)).astype(f16)
    b2c = np.asarray(b2, f32).reshape(128, 1).copy()
    g1c = np.asarray(ln1_g, f32).reshape(128, 1).copy()
    b1lc = np.asarray(ln1_b, f32).reshape(128, 1).copy()
    g2c = np.asarray(ln2_g, f32).reshape(128, 1).copy()
    b2lc = np.asarray(ln2_b, f32).reshape(128, 1).copy()

    shared = dict(wcat=wcat, adst=adst, arep=arep, sel=sel, eye8=eye8, e16=e16,
                  w1c=w1c, b1c=b1c, w2c=w2c, b2c=b2c, g1c=g1c, b1lc=b1lc,
                  g2c=g2c, b2lc=b2lc)

    h = np.asarray(h, f32)
    adj = np.asarray(adj_mask)
    in_maps = []
    for b in range(B):
        hT = np.ascontiguousarray(h[b].T).astype(f16)                  # [128, 1024]
        adjT = np.ascontiguousarray(
            (adj[b] != 0).T.astype(f16).reshape(NB, 128, N).transpose(1, 0, 2).reshape(128, NB * N)
        )
        in_maps.append(dict(hT=hT, adjT=adjT, **shared))
    return in_maps


def kernel(**inputs):
    from concourse.bass_utils import run_bass_kernel_spmd

    if "nc" not in _CACHE:
        _CACHE["nc"] = _build_program()
    nc = _CACHE["nc"]

    in_maps = _host_prep(**inputs)
    res = run_bass_kernel_spmd(nc, in_maps, list(range(B)))
    out = np.empty((B, N, OUT_DIM), np.float32)
    for b in range(B):
        out[b] = res.results[b]["outT"].T
    return out


# revision 12
# speedup vs baseline: 1.8492x; 1.0978x over previous
"""MultiHeadGAT Trainium2 kernel: 8-core batch-parallel, transposed-layout pipeline.

Math: for scores e = lrelu(s_i[n] + s_j[m]), softmax numerator
  p = exp(lrelu(s_i+s_j)) = e^{0.2 s_i} * max(e^{0.8 s_i} * e^{s_j}, e^{0.2 s_j})
The e^{0.2 s_i} row factor cancels in softmax, so on-device we only compute
  q[m, n] = adjT[m, n] * max(Wbc[m, n] * u[m], v[m])
with Wbc = broadcast(e^{0.8 s_i}) (n-varying), u = e^{s_j}, v = e^{0.2 s_j}
(per-partition scalars), which is one fused tensor_scalar (mult+max) plus one
tensor_tensor (mask) per tile. Attention output and row-sum Z come from one
PE matmul with lhsT = [ones | pad | Wh_head]; normalization 1/Z = exp(-ln(Z)).
"""

import sys

sys.path.insert(0, "/opt/trn_rl_repo")

import numpy as np

B, N, IN_DIM, H, HD = 8, 1024, 128, 8, 16
OUT_DIM = H * HD
EPS = 1e-5
NB = N // 128  # 8 m-blocks

_CACHE = {}


def _patch_act_tables():
    # Force one activation table set for the whole kernel: every function we
    # use (Exp, Ln, Copy, Square, Relu, Identity) lives in
    # natural_log_exp_and_others; emptying the other sets makes Bacc's
    # table-load inserter emit exactly one ACT_TABLE_LOAD instead of
    # thrashing between exp/ln/small sets (~2.5us per reload).
    import concourse.bacc as bacc
    import concourse.hw_specs as hw_specs
    if getattr(bacc, "_act_tables_patched", False):
        return
    orig = hw_specs.get_activation_tables

    def patched(arch):
        t = dict(orig(arch))
        keep = "natural_log_exp_and_others"
        return {k: (v if k == keep else set()) for k, v in t.items()}

    bacc.get_activation_tables = patched
    bacc._act_tables_patched = True


def _build_program():
    import concourse.bacc as bacc
    import concourse.mybir as mybir
    import concourse.tile as tile

    _patch_act_tables()

    F16 = mybir.dt.float16
    F32 = mybir.dt.float32
    AF = mybir.ActivationFunctionType
    OP = mybir.AluOpType

    nc = bacc.Bacc("TRN2", target_bir_lowering=False, debug=False, num_devices=8)

    # ---- I/O ----
    hT = nc.dram_tensor("hT", [128, N], F16, kind="ExternalInput")
    adjT = nc.dram_tensor("adjT", [128, NB * N], F16, kind="ExternalInput")
    wcat = nc.dram_tensor("wcat", [128, 128], F16, kind="ExternalInput")
    adst = nc.dram_tensor("adst", [128, H], F16, kind="ExternalInput")
    arep = nc.dram_tensor("arep", [128, H * 128], F16, kind="ExternalInput")
    sel = nc.dram_tensor("sel", [16, H * 128], F16, kind="ExternalInput")
    e16cat = nc.dram_tensor("e16cat", [1, H * 128], F16, kind="ExternalInput")
    w1c = nc.dram_tensor("w1c", [128, 2 * OUT_DIM], F16, kind="ExternalInput")
    b1c = nc.dram_tensor("b1c", [128, 2], F32, kind="ExternalInput")
    w2c = nc.dram_tensor("w2c", [128, 2 * OUT_DIM], F16, kind="ExternalInput")
    b2c = nc.dram_tensor("b2c", [128, 1], F32, kind="ExternalInput")
    g1c = nc.dram_tensor("g1c", [128, 1], F32, kind="ExternalInput")
    b1lc = nc.dram_tensor("b1lc", [128, 1], F32, kind="ExternalInput")
    g2c = nc.dram_tensor("g2c", [128, 1], F32, kind="ExternalInput")
    b2lc = nc.dram_tensor("b2lc", [128, 1], F32, kind="ExternalInput")
    outT = nc.dram_tensor("outT", [128, N], F32, kind="ExternalOutput")

    with tile.TileContext(nc) as tc:
        with (
            tc.tile_pool(name="const", bufs=1) as cpool,
            tc.tile_pool(name="big", bufs=1) as big,
            tc.tile_pool(name="work", bufs=2) as work,
            tc.tile_pool(name="mid", bufs=1) as mid,
            tc.tile_pool(name="rows", bufs=1) as rows,
        ):
            # ---- load everything ----
            hT_t = cpool.tile([128, N], F16)
            nc.sync.dma_start(hT_t[:], hT[:])
            wcat_t = cpool.tile([128, 128], F16)
            nc.sync.dma_start(wcat_t[:], wcat[:])
            adst_t = cpool.tile([128, H], F16)
            nc.sync.dma_start(adst_t[:], adst[:])
            arep_t = cpool.tile([128, H * 128], F16)
            nc.sync.dma_start(arep_t[:], arep[:])
            sel_t = cpool.tile([16, H * 128], F16)
            nc.sync.dma_start(sel_t[:], sel[:])
            e16cat_t = cpool.tile([1, H * 128], F16)
            nc.sync.dma_start(e16cat_t[:], e16cat[:])
            w1_t = cpool.tile([128, 2 * OUT_DIM], F16)
            nc.sync.dma_start(w1_t[:], w1c[:])
            b1_t = cpool.tile([128, 2], F32)
            nc.sync.dma_start(b1_t[:], b1c[:])
            w2_t = cpool.tile([128, 2 * OUT_DIM], F16)
            nc.sync.dma_start(w2_t[:], w2c[:])
            b2_t = cpool.tile([128, 1], F32)
            nc.sync.dma_start(b2_t[:], b2c[:])
            g1_t = cpool.tile([128, 1], F32)
            nc.sync.dma_start(g1_t[:], g1c[:])
            b1l_t = cpool.tile([128, 1], F32)
            nc.sync.dma_start(b1l_t[:], b1lc[:])
            g2_t = cpool.tile([128, 1], F32)
            nc.sync.dma_start(g2_t[:], g2c[:])
            b2l_t = cpool.tile([128, 1], F32)
            nc.sync.dma_start(b2l_t[:], b2lc[:])

            adjT_t = cpool.tile([128, NB * N], F16)
            nc.sync.dma_start(adjT_t[:], adjT[:])

            onescol = cpool.tile([128, 1], F16)
            nc.vector.memset(onescol[:], 1.0)
            jmat = cpool.tile([128, 128], F16)
            nc.vector.memset(jmat[:], 1.0 / 128)
            zbias = cpool.tile([128, 1], F32)
            nc.vector.memset(zbias[:], 1e-4)
            epsbias = cpool.tile([128, 1], F32)
            nc.vector.memset(epsbias[:], EPS)
            onesrow = cpool.tile([1, 128], F32)
            nc.vector.memset(onesrow[:], 1.0)

            # ---- phase 1: WhT, Wh_nat->aug, s-cols(u,v), Wbc ----
            whT_sb = big.tile([128, N], F16)     # Wh^T: [(h,d), n]
            whnat_sb = big.tile([128, NB * 128], F16)  # Wh natural per m-block
            aug = big.tile([128, NB * 384], F16)  # per (mb,h): [ones|0*31|Wh16]
            u_all = big.tile([128, NB * H], F32)
            v_all = big.tile([128, NB * H], F32)
            wbc = [big.tile([128, N], F16, tag=f"wbc{i}", name=f"wbc{i}") for i in range(H)]

            with tc.tile_pool(name="ps1", bufs=3, space="PSUM") as ps1:
                whT_ps = ps1.tile([128, N], F32, tag="ps1")
                for ch in range(2):
                    nc.tensor.matmul(
                        whT_ps[:, ch * 512:(ch + 1) * 512], wcat_t[:],
                        hT_t[:, ch * 512:(ch + 1) * 512], start=True, stop=True,
                    )
                nc.scalar.activation(whT_sb[:], whT_ps[:], AF.Copy)

                # aug skeleton: zeros + ones columns
                nc.vector.memset(aug[:], 0.0)
                aug4 = aug[:].rearrange("p (m h c) -> p m h c", m=NB, h=H, c=48)
                nc.vector.memset(aug4[:, :, :, 0:1], 1.0)

                for mb in range(NB):
                    sc_ps = ps1.tile([128, H], F32, tag="ps1")
                    nc.tensor.matmul(
                        sc_ps[:], whT_sb[:, mb * 128:(mb + 1) * 128], adst_t[:],
                        start=True, stop=True,
                    )
                    nc.scalar.activation(
                        u_all[:, mb * H:(mb + 1) * H], sc_ps[:], AF.Exp, scale=1.0
                    )
                    nc.scalar.activation(
                        v_all[:, mb * H:(mb + 1) * H], sc_ps[:], AF.Exp, scale=0.2
                    )
                for hh in range(H):
                    wb_ps = ps1.tile([128, N], F32, tag="ps1")
                    for ch in range(2):
                        nc.tensor.matmul(
                            wb_ps[:, ch * 512:(ch + 1) * 512],
                            arep_t[:, hh * 128:(hh + 1) * 128],
                            whT_sb[:, ch * 512:(ch + 1) * 512],
                            start=True, stop=True,
                        )
                    nc.scalar.activation(wbc[hh][:], wb_ps[:], AF.Exp, scale=0.8)
                for mb in range(NB):
                    wn_ps = ps1.tile([128, 128], F32, tag="ps1")
                    nc.tensor.matmul(
                        wn_ps[:], hT_t[:, mb * 128:(mb + 1) * 128], wcat_t[:],
                        start=True, stop=True,
                    )
                    nc.scalar.activation(
                        whnat_sb[:, mb * 128:(mb + 1) * 128], wn_ps[:], AF.Copy
                    )
                for mb in range(NB):
                    for hh in range(H):
                        nc.scalar.activation(
                            aug[:, mb * 384 + hh * 48 + 32: mb * 384 + hh * 48 + 48],
                            whnat_sb[:, mb * 128 + hh * 16: mb * 128 + hh * 16 + 16],
                            AF.Copy,
                        )

            # ---- phase 2: attention ----
            stage_all = big.tile([16, H * N], F16)
            with (
                tc.tile_pool(name="ps48", bufs=2, space="PSUM") as ps48,
                tc.tile_pool(name="psacc", bufs=1, space="PSUM") as psacc,
                tc.tile_pool(name="psz", bufs=1, space="PSUM") as psz,
            ):
                asm_ps = psacc.tile([128, N], F32)
                zbc_ps = psz.tile([128, N], F32)
                for hh in range(H):
                    q_all = work.tile([128, NB * N], F16, tag="q")
                    t1 = work.tile([128, NB * N], F16, tag="t1")
                    for mb in range(NB):
                        nc.vector.tensor_scalar(
                            t1[:, mb * N:(mb + 1) * N],
                            wbc[hh][:],
                            u_all[:, mb * H + hh: mb * H + hh + 1],
                            v_all[:, mb * H + hh: mb * H + hh + 1],
                            op0=OP.mult, op1=OP.max,
                        )
                    for half in range(2):
                        nc.vector.tensor_tensor(
                            q_all[:, half * 4 * N:(half + 1) * 4 * N],
                            t1[:, half * 4 * N:(half + 1) * 4 * N],
                            adjT_t[:, half * 4 * N:(half + 1) * 4 * N], op=OP.mult,
                        )
                    at_ps = ps48.tile([48, N], F32)
                    for mb in range(NB):
                        for ch in range(2):
                            nc.tensor.matmul(
                                at_ps[:, ch * 512:(ch + 1) * 512],
                                aug[:, mb * 384 + hh * 48: mb * 384 + hh * 48 + 48],
                                q_all[:, mb * N + ch * 512: mb * N + ch * 512 + 512],
                                start=(mb == 0), stop=(mb == NB - 1),
                            )
                    nc.scalar.activation(
                        stage_all[:, hh * N:(hh + 1) * N], at_ps[32:48, :], AF.Copy
                    )
                    lnz = work.tile([1, N], F32, tag="lnz")
                    nc.scalar.activation(lnz[:], at_ps[0:1, :], AF.Ln, bias=zbias[0:1, :])
                    zinv_h = work.tile([1, N], F16, tag="zinvh")
                    nc.scalar.activation(zinv_h[:], lnz[:], AF.Exp, scale=-1.0)
                    for ch in range(2):
                        nc.tensor.matmul(
                            zbc_ps[:, ch * 512:(ch + 1) * 512],
                            e16cat_t[0:1, hh * 128:(hh + 1) * 128],
                            zinv_h[0:1, ch * 512:(ch + 1) * 512],
                            start=(hh == 0), stop=(hh == H - 1),
                        )
                        nc.tensor.matmul(
                            asm_ps[:, ch * 512:(ch + 1) * 512],
                            sel_t[:, hh * 128:(hh + 1) * 128],
                            stage_all[:, hh * N + ch * 512: hh * N + ch * 512 + 512],
                            start=(hh == 0), stop=(hh == H - 1),
                        )

                stage_full = big.tile([128, N], F16)
                nc.scalar.activation(stage_full[:], asm_ps[:], AF.Copy)
                zbcf = big.tile([128, N], F16)
                nc.scalar.activation(zbcf[:], zbc_ps[:], AF.Copy)

            with tc.tile_pool(name="ps3", bufs=2, space="PSUM") as ps3:
                # ---- chunked epilogue: normalize+residual, LN1, FFN, LN2 ----
                C = 512

                def cs(t, c):
                    return t[:, c * C:(c + 1) * C]

                hh_t = big.tile([128, N], F16)
                x_res = big.tile([128, N], F16)

                def layernorm_T(x_in, g_col, b_col, out_tile, ps_pool, nm):
                    """Column-chunked transposed layernorm; J=ones/128 matmul
                    produces mean / mean-square directly as broadcast tiles."""
                    x2 = mid.tile([128, N], F16, tag=f"x2{nm}")
                    for c in range(2):
                        nc.vector.tensor_tensor(
                            cs(x2, c), cs(x_in, c), cs(x_in, c), op=OP.mult
                        )
                    for c in range(2):
                        mu_ps = ps_pool.tile([128, C], F32, tag="psb")
                        ssq_ps = ps_pool.tile([128, C], F32, tag="psb")
                        nc.tensor.matmul(mu_ps[:], jmat[:], cs(x_in, c),
                                         start=True, stop=True)
                        nc.tensor.matmul(ssq_ps[:], jmat[:], cs(x2, c),
                                         start=True, stop=True)
                        mu_bc = mid.tile([128, C], F16, tag=f"mbc{nm}{c}")
                        nc.scalar.activation(mu_bc[:], mu_ps[:], AF.Copy)
                        ssq_bc = mid.tile([128, C], F16, tag=f"sbc{nm}{c}")
                        nc.scalar.activation(ssq_bc[:], ssq_ps[:], AF.Copy)
                        mu2 = mid.tile([128, C], F16, tag=f"m2{nm}{c}")
                        nc.vector.tensor_tensor(mu2[:], mu_bc[:], mu_bc[:], op=OP.mult)
                        var = mid.tile([128, C], F16, tag=f"va{nm}{c}")
                        nc.vector.tensor_tensor(var[:], ssq_bc[:], mu2[:],
                                                op=OP.subtract)
                        lnv = mid.tile([128, C], F16, tag=f"lv{nm}{c}")
                        nc.scalar.activation(lnv[:], var[:], AF.Ln, bias=epsbias[:])
                        rstd = mid.tile([128, C], F16, tag=f"rs{nm}{c}")
                        nc.scalar.activation(rstd[:], lnv[:], AF.Exp, scale=-0.5)
                        t_ = mid.tile([128, C], F16, tag=f"lnt{nm}{c}")
                        nc.vector.tensor_tensor(t_[:], cs(x_in, c), mu_bc[:],
                                                op=OP.subtract)
                        xn = mid.tile([128, C], F16, tag=f"lnxn{nm}{c}")
                        nc.vector.tensor_tensor(xn[:], t_[:], rstd[:], op=OP.mult)
                        nc.vector.tensor_scalar(
                            cs(out_tile, c), xn[:], g_col[:], b_col[:],
                            op0=OP.mult, op1=OP.add,
                        )

                xc = big.tile([128, N], F16)
                y1s = big.tile([128, 2 * N], F16)
                y2b = big.tile([128, N], F16)
                z_res = big.tile([128, N], F16)
                outT_sb = big.tile([128, N], F32)

                for c in range(2):
                    nc.vector.tensor_tensor(cs(hh_t, c), cs(stage_full, c),
                                            cs(zbcf, c), op=OP.mult)
                    nc.vector.tensor_tensor(cs(x_res, c), cs(hh_t, c),
                                            cs(hT_t, c), op=OP.add)
                layernorm_T(x_res, g1_t, b1l_t, xc, ps3, "a")

                # FFN (chunked)
                for cb in range(2):
                    y1_ps = ps3.tile([128, N], F32, tag="ps3")
                    for c in range(2):
                        nc.tensor.matmul(
                            cs(y1_ps, c), w1_t[:, cb * 128:(cb + 1) * 128],
                            cs(xc, c), start=True, stop=True,
                        )
                        nc.scalar.activation(
                            y1s[:, cb * N + c * C: cb * N + (c + 1) * C],
                            cs(y1_ps, c), AF.Relu, bias=b1_t[:, cb:cb + 1],
                        )
                y2_ps = ps3.tile([128, N], F32, tag="ps3")
                for cb in range(2):
                    for c in range(2):
                        nc.tensor.matmul(
                            cs(y2_ps, c), w2_t[:, cb * 128:(cb + 1) * 128],
                            y1s[:, cb * N + c * C: cb * N + (c + 1) * C],
                            start=(cb == 0), stop=(cb == 1),
                        )
                for c in range(2):
                    nc.scalar.activation(cs(y2b, c), cs(y2_ps, c), AF.Identity,
                                         bias=b2_t[:])
                    nc.vector.tensor_tensor(cs(z_res, c), cs(y2b, c), cs(xc, c),
                                            op=OP.add)
                layernorm_T(z_res, g2_t, b2l_t, outT_sb, ps3, "b")
                for c in range(2):
                    nc.sync.dma_start(outT[:, c * C:(c + 1) * C],
                                      outT_sb[:, c * C:(c + 1) * C])

    nc.compile()
    return nc


def _host_prep(h, adj_mask, W, a, ln1_g, ln1_b, w1, b1, w2, b2, ln2_g, ln2_b):
    f16 = np.float16
    f32 = np.float32
    # shared weights
    wcat = np.ascontiguousarray(np.transpose(np.asarray(W, f32), (1, 0, 2)).reshape(128, 128)).astype(f16)
    a = np.asarray(a, f32)
    a_src, a_dst = a[:, :HD], a[:, HD:]
    adst = np.zeros((128, H), f16)
    arep = np.zeros((128, H * 128), f16)
    for hh in range(H):
        adst[hh * HD:(hh + 1) * HD, hh] = a_dst[hh].astype(f16)
        arep[hh * HD:(hh + 1) * HD, hh * 128:(hh + 1) * 128] = (
            a_src[hh].astype(f16)[:, None]
        )
    sel = np.zeros((16, H * 128), f16)
    for hh in range(H):
        sel[np.arange(16), hh * 128 + hh * 16 + np.arange(16)] = 1.0
    e16cat = np.zeros((1, H * 128), f16)
    for hh in range(H):
        e16cat[0, hh * 128 + hh * 16: hh * 128 + (hh + 1) * 16] = 1.0
    w1c = np.asarray(w1, f32).astype(f16)                      # [128, 256]
    b1c = np.asarray(b1, f32).reshape(2, 128).T.copy()          # [128, 2]
    w2c = np.ascontiguousarray(np.asarray(w2, f32).reshape(2, 128, 128).transpose(1, 0, 2).reshape(128, 256)).astype(f16)
    b2c = np.asarray(b2, f32).reshape(128, 1).copy()
    g1c = np.asarray(ln1_g, f32).reshape(128, 1).copy()
    b1lc = np.asarray(ln1_b, f32).reshape(128, 1).copy()
    g2c = np.asarray(ln2_g, f32).reshape(128, 1).copy()
    b2lc = np.asarray(ln2_b, f32).reshape(128, 1).copy()

    shared = dict(wcat=wcat, adst=adst, arep=arep, sel=sel, e16cat=e16cat,
                  w1c=w1c, b1c=b1c, w2c=w2c, b2c=b2c, g1c=g1c, b1lc=b1lc,
                  g2c=g2c, b2lc=b2lc)

    h = np.asarray(h, f32)
    adj = np.asarray(adj_mask)
    in_maps = []
    for b in range(B):
        hT = np.ascontiguousarray(h[b].T).astype(f16)                  # [128, 1024]
        adjT = np.ascontiguousarray(
            (adj[b] != 0).T.astype(f16).reshape(NB, 128, N).transpose(1, 0, 2).reshape(128, NB * N)
        )
        in_maps.append(dict(hT=hT, adjT=adjT, **shared))
    return in_maps


def kernel(**inputs):
    from concourse.bass_utils import run_bass_kernel_spmd

    if "nc" not in _CACHE:
        _CACHE["nc"] = _build_program()
    nc = _CACHE["nc"]

    in_maps = _host_prep(**inputs)
    res = run_bass_kernel_spmd(nc, in_maps, list(range(B)))
    out = np.empty((B, N, OUT_DIM), np.float32)
    for b in range(B):
        out[b] = res.results[b]["outT"].T
    return out
